# revision 10
# baseline (speedup 1.0000x reference)
"""APPNP (GNN message passing) distributed Bass kernel for 8 TRN2 NeuronCores.

Strategy (graph/data parallel, "pull" form):
  - Nodes sharded by contiguous id range across 8 cores (12500 each).
  - MLP head computed on-device per shard (xT passed pre-transposed bf16).
  - Each propagation step: AllGather the full f32 h table (100000 x 64) into
    per-core DRAM, then each core gathers h[src] rows for its dst-range edges
    via dma_gather (int16 indices -> table split into 4 row-quarters), scales
    by edge norm via a weighted one-hot (DVE is_equal + mul), and scatter-adds
    into per-tile PSUM accumulators with TensorEngine matmuls
    (agg_tile += S'^T @ msg).  Teleport term enters PSUM as an extra
    identity-matmul with 0.1*h0 (bf16).
  - Edge structure (sort, cells, padding, norms) is preprocessed on host;
    slot layout is identical on all 8 cores (SPMD single graph).
"""

import sys

sys.path.insert(0, "/opt/trn_rl_repo")

import numpy as np
import concourse.bass as bass
import concourse.bacc as bacc
import concourse.tile as tile
import concourse.mybir as mybir
from concourse import bass_utils

F32 = mybir.dt.float32
BF16 = mybir.dt.bfloat16
I16 = mybir.dt.int16
I32 = mybir.dt.int32
AF = mybir.ActivationFunctionType
ALU = mybir.AluOpType

P = 128


class Cfg:
    def __init__(self, N=100000, E=3200000, IN=512, HID=256, OUT=64, K=10,
                 alpha=0.1, NC=8, NQ=4, TB=4):
        self.N, self.E, self.IN, self.HID, self.OUT, self.K = N, E, IN, HID, OUT, K
        self.alpha, self.NC, self.NQ = alpha, NC, NQ
        self.SH = N // NC                      # nodes per core
        self.NT = (self.SH + P - 1) // P       # dst tiles per core
        self.PSH = self.NT * P                 # padded shard rows
        self.QR = N // NQ                      # table rows per quarter (int16 range)
        assert N % NC == 0 and N % NQ == 0 and self.QR <= 32767
        self.TB = TB                           # tiles per batch
        self.NB = (self.NT + TB - 1) // TB     # batches
        self.CT = 2                            # tiles per gather call
        self.SCRATCH = 49152                   # swdge ring bytes (ring = /16 descs)
        self.KI = IN // P                      # k chunks of layer 1
        self.KH = HID // P                     # k chunks of layer 2
        self.RB = 512                          # mlp row batch
        assert IN % P == 0 and HID % P == 0 and OUT <= P


MAXG = 1024  # hw swdge ring: max descriptors per dma_gather call


def gather_pieces(cfg, cell, off):
    """Per-batch list of gather calls [(q, slot_off, n_slots)], each <= MAXG."""
    NT, NQ = cfg.NT, cfg.NQ
    out = []
    for b in range(cfg.NB):
        bts = list(range(b * cfg.TB, min((b + 1) * cfg.TB, NT)))
        pieces = []
        for g0 in range(0, len(bts), cfg.CT):
            gts = bts[g0:g0 + cfg.CT]
            for q in range(NQ):
                o = int(off[gts[0], q])
                n = int(cell[gts[0]:gts[-1] + 1, q].sum())
                p = o
                while p < o + n:
                    m = min(MAXG, o + n - p)
                    pieces.append((q, p, m))
                    p += m
        out.append(pieces)
    return out


def preprocess(cfg, x, edge_index, W1, b1, W2, b2):
    """Host-side graph preprocessing. Returns (static schedule, per-core in_maps)."""
    N, NC, SH, NT, NQ, QR = cfg.N, cfg.NC, cfg.SH, cfg.NT, cfg.NQ, cfg.QR
    src = np.concatenate([edge_index[0], np.arange(N, dtype=np.int64)])
    dst = np.concatenate([edge_index[1], np.arange(N, dtype=np.int64)])
    deg = np.bincount(dst, minlength=N).astype(np.float64)
    dinv = 1.0 / np.sqrt(deg)
    wall = ((1.0 - cfg.alpha) * dinv[src] * dinv[dst]).astype(np.float32)

    cores = []
    cnts = np.zeros((NC, NT, NQ), np.int64)
    for c in range(NC):
        sel = (dst >= c * SH) & (dst < (c + 1) * SH)
        s_c = src[sel]
        d_c = (dst[sel] - c * SH).astype(np.int64)
        w_c = wall[sel]
        t = d_c // P
        q = s_c // QR
        o = np.lexsort((d_c, q, t))
        s_c, d_c, w_c, t, q = s_c[o], d_c[o], w_c[o], t[o], q[o]
        cnts[c] = np.bincount(t * NQ + q, minlength=NT * NQ).reshape(NT, NQ)
        cores.append((s_c, d_c, w_c))

    # static per-(tile, quarter) slot counts: max over cores, rounded to 128
    cell = ((cnts.max(axis=0) + P - 1) // P * P).astype(np.int64)  # [NT, NQ]
    cell = np.maximum(cell, P)  # at least one chunk so matmul groups are non-empty
    TOT = int(cell.sum())

    # slot offsets in (batch -> quarter -> tile) order
    off = np.zeros((NT, NQ), np.int64)
    pos = 0
    for b in range(cfg.NB):
        bts = list(range(b * cfg.TB, min((b + 1) * cfg.TB, NT)))
        for g0 in range(0, len(bts), cfg.CT):
            gts = bts[g0:g0 + cfg.CT]
            for q in range(NQ):
                for t in gts:
                    off[t, q] = pos
                    pos += int(cell[t, q])
    assert pos == TOT

    in_maps = []
    for c in range(NC):
        s_c, d_c, w_c = cores[c]
        gidx = np.zeros(TOT, np.int16)
        gw = np.zeros(TOT, np.float32)
        gdl = np.zeros(TOT, np.float32)
        p0 = 0
        for t in range(NT):
            for q in range(NQ):
                n = int(cnts[c, t, q])
                o = off[t, q]
                gidx[o:o + n] = (s_c[p0:p0 + n] - q * QR).astype(np.int16)
                gw[o:o + n] = w_c[p0:p0 + n]
                gdl[o:o + n] = (d_c[p0:p0 + n] % P).astype(np.float32)
                p0 += n
        assert p0 == len(s_c)
        # idx wrap layout [16, TOT/16] per call, replicated to 128 partitions.
        # call = (batch, quarter) -> contiguous slot range
        gidx16 = np.zeros((16, TOT // 16), np.int16)
        for pieces in gather_pieces(cfg, cell, off):
            for (q, o, n) in pieces:
                blk = gidx[o:o + n].reshape(n // 16, 16).T  # j -> [j%16, j//16]
                gidx16[:, o // 16:(o + n) // 16] = blk
        gidx128 = np.tile(gidx16, (8, 1))  # replicate for 8 q7 cores
        # w / dstlocal in gather-out layout: slot j -> [j%128, j//128]
        gw128 = gw.reshape(TOT // P, P).T.astype(np.float32)
        gdl128 = gdl.reshape(TOT // P, P).T.astype(np.float32)
        bf16 = mybir.dt.np(mybir.dt.bfloat16)
        to_bf16 = lambda a: np.asarray(a, np.float32).astype(bf16)
        x_c = np.zeros((cfg.PSH, cfg.IN), np.float32)
        x_c[:SH] = x[c * SH:(c + 1) * SH]
        in_maps.append({
            "xt": to_bf16(np.ascontiguousarray(x_c.T)),
            "w1": to_bf16(W1),
            "b1": np.ascontiguousarray(b1.reshape(cfg.HID // P, P).T.astype(np.float32)),
            "w2": to_bf16(W2),
            "b2": to_bf16(b2.reshape(1, cfg.OUT)),
            "gidx": gidx128,
            "gw": to_bf16(gw128),
            "gdl": to_bf16(gdl128),
        })
    sched = {"cell": cell, "off": off, "TOT": TOT}
    return sched, in_maps


def build(cfg, sched):
    cell, off, TOT = sched["cell"], sched["off"], sched["TOT"]
    N, NT, NQ, QR, OUT, SH = cfg.N, cfg.NT, cfg.NQ, cfg.QR, cfg.OUT, cfg.SH
    NB, TB, K = cfg.NB, cfg.TB, cfg.K

    nc = bacc.Bacc("TRN2", target_bir_lowering=False, debug=False, num_devices=cfg.NC,
                   dynamic_dma_scratch_size=cfg.SCRATCH)

    xt_d = nc.declare_dram_parameter("xt", [cfg.IN, cfg.PSH], BF16, isOutput=False)
    w1_d = nc.declare_dram_parameter("w1", [cfg.IN, cfg.HID], BF16, isOutput=False)
    b1_d = nc.declare_dram_parameter("b1", [P, cfg.KH], F32, isOutput=False)
    w2_d = nc.declare_dram_parameter("w2", [cfg.HID, OUT], BF16, isOutput=False)
    b2_d = nc.declare_dram_parameter("b2", [1, OUT], BF16, isOutput=False)
    gidx_d = nc.declare_dram_parameter("gidx", [P, TOT // 16], I16, isOutput=False)
    gw_d = nc.declare_dram_parameter("gw", [P, TOT // P], BF16, isOutput=False)
    gdl_d = nc.declare_dram_parameter("gdl", [P, TOT // P], BF16, isOutput=False)
    out_d = nc.declare_dram_parameter("out", [SH, OUT], F32, isOutput=True)

    table = nc.dram_tensor("table", [N, OUT], F32, addr_space="Shared")
    send = nc.dram_tensor("send", [SH, OUT], F32)

    # per-batch call sizes (slots) and chunk schedule
    allpieces = gather_pieces(cfg, cell, off)
    batches = []
    for b in range(NB):
        ts = list(range(b * TB, min((b + 1) * TB, NT)))
        regions = []   # (q, slot_off, n): contiguous (group, quarter) spans for cast/S
        chunks = []
        col = 0
        for g0 in range(0, len(ts), cfg.CT):
            gts = ts[g0:g0 + cfg.CT]
            for q in range(NQ):
                n = int(cell[gts[0]:gts[-1] + 1, q].sum())
                regions.append((q, int(off[gts[0], q]), n))
                for t in gts:
                    for j in range(int(cell[t, q]) // P):
                        chunks.append((col, t - b * TB))
                        col += 1
        last_of_tile = {}
        for k_, (col_, i) in enumerate(chunks):
            last_of_tile[i] = k_
        batches.append((ts, allpieces[b], regions, chunks, last_of_tile))
    maxreg = max(n for (_, _, regions, _, _) in batches for (_, _, n) in regions)
    maxbatch = max(sum(n for (_, _, n) in regions) for (_, _, regions, _, _) in batches)

    with tile.TileContext(nc) as tc:
        with tc.tile_pool(name="const", bufs=1) as cp:
            iota = cp.tile([P, P], BF16, tag="iota")
            iota_i = cp.tile([P, P], I32, tag="iota_i")
            nc.gpsimd.iota(iota_i[:], pattern=[[1, P]], base=0, channel_multiplier=0)
            nc.vector.tensor_copy(iota[:], iota_i[:])
            iop = cp.tile([P, 1], I32, tag="iop")
            nc.gpsimd.iota(iop[:], pattern=[[0, 1]], base=0, channel_multiplier=1)
            iopb = cp.tile([P, 1], BF16, tag="iopb")
            nc.vector.tensor_copy(iopb[:], iop[:])
            ident = cp.tile([P, P], BF16, tag="ident")
            nc.vector.tensor_tensor(
                out=ident[:], in0=iopb[:].to_broadcast([P, P]), in1=iota[:],
                op=ALU.is_equal)
            ones = cp.tile([1, P], BF16, tag="ones")
            nc.gpsimd.memset(ones[:], 1.0)

            w1s = cp.tile([P, cfg.KI, cfg.HID], BF16, tag="w1s")
            nc.sync.dma_start(w1s[:], w1_d.ap().rearrange("(k p) h -> p k h", p=P))
            w2s = cp.tile([P, cfg.KH, OUT], BF16, tag="w2s")
            nc.sync.dma_start(w2s[:], w2_d.ap().rearrange("(k p) o -> p k o", p=P))
            b1s = cp.tile([P, cfg.KH], F32, tag="b1s")
            nc.sync.dma_start(b1s[:], b1_d[:, :])
            b2s = cp.tile([1, OUT], BF16, tag="b2s")
            nc.sync.dma_start(b2s[:], b2_d[:, :])
            gw = cp.tile([P, TOT // P], BF16, tag="gw")
            nc.sync.dma_start(gw[:], gw_d[:, :])
            gdl = cp.tile([P, TOT // P], BF16, tag="gdl")
            nc.sync.dma_start(gdl[:], gdl_d[:, :])

            h0t = cp.tile([P, NT * OUT], BF16, tag="h0t")    # 0.1 * h0, bf16
            hnew = cp.tile([P, NT * OUT], F32, tag="hnew")   # current h, f32

            # ---------------- MLP head ----------------
            with (
                tc.tile_pool(name="mxt", bufs=2 * cfg.KI) as mxt,
                tc.tile_pool(name="mp1", bufs=2, space="PSUM") as mp1,
                tc.tile_pool(name="mh1", bufs=2) as mh1,
                tc.tile_pool(name="mp2", bufs=2, space="PSUM") as mp2,
            ):
                RB = cfg.RB
                nrb = (cfg.PSH + RB - 1) // RB
                for b in range(nrb):
                    r0 = b * RB
                    rows = min(RB, cfg.PSH - r0)
                    xts = []
                    for k_ in range(cfg.KI):
                        xk = mxt.tile([P, RB], BF16, tag="xt")
                        nc.sync.dma_start(xk[:, :rows], xt_d[k_ * P:(k_ + 1) * P, r0:r0 + rows])
                        xts.append(xk)
                    h1t = mh1.tile([P, cfg.KH, RB], BF16, tag="h1t")
                    for fc in range(cfg.KH):
                        ps = mp1.tile([P, RB], F32, tag=f"mp1_{fc}")
                        for k_ in range(cfg.KI):
                            nc.tensor.matmul(
                                ps[:, :rows],
                                lhsT=w1s[:, k_, fc * P:(fc + 1) * P],
                                rhs=xts[k_][:, :rows],
                                start=(k_ == 0), stop=(k_ == cfg.KI - 1))
                        nc.scalar.activation(
                            h1t[:, fc, :rows], ps[:, :rows], AF.Relu,
                            bias=b1s[:, fc:fc + 1], scale=1.0)
                    for tl in range(rows // P):
                        gt = r0 // P + tl
                        p2 = mp2.tile([P, OUT], F32, tag="mp2")
                        for fc in range(cfg.KH):
                            nc.tensor.matmul(
                                p2[:], lhsT=h1t[:, fc, tl * P:(tl + 1) * P],
                                rhs=w2s[:, fc, :], start=(fc == 0), stop=False)
                        nc.tensor.matmul(p2[:], lhsT=ones[:1, :], rhs=b2s[:1, :],
                                         start=False, stop=True)
                        nc.scalar.activation(h0t[:, gt * OUT:(gt + 1) * OUT], p2[:],
                                             AF.Copy, bias=0.0, scale=cfg.alpha)
                        nc.vector.tensor_copy(hnew[:, gt * OUT:(gt + 1) * OUT], p2[:])

            def send_h():
                full = (NT - 1) * P
                nc.sync.dma_start(
                    send.ap()[0:full, :].rearrange("(t p) d -> p t d", p=P),
                    hnew[:, 0:(NT - 1) * OUT].rearrange("p (t d) -> p t d", d=OUT))
                rem = SH - full
                nc.sync.dma_start(
                    send[full:SH, :],
                    hnew[0:rem, (NT - 1) * OUT:NT * OUT])

            def all_gather():
                nc.gpsimd.collective_compute(
                    "AllGather", ALU.bypass,
                    ins=[send.ap().opt()], outs=[table.ap().opt()],
                    replica_groups=[list(range(cfg.NC))])

            send_h()
            all_gather()

            # ---------------- propagation ----------------
            with (
                tc.tile_pool(name="ixp", bufs=2) as ixp,
                tc.tile_pool(name="gbp", bufs=3) as gbp,
                tc.tile_pool(name="msgp", bufs=2) as msgp,
                tc.tile_pool(name="sp", bufs=3) as sp,
                tc.tile_pool(name="pp", bufs=2, space="PSUM") as pp,
            ):
                for s in range(K):
                    for (ts, pieces, regions, chunks, last_of_tile) in batches:
                        b0 = int(off[ts[0], 0])           # first slot of batch
                        cols_b = sum(n for (_, _, n) in regions)
                        ix = ixp.tile([P, maxbatch // 16], I16, tag="ix")
                        nc.sync.dma_start(ix[:, :cols_b // 16],
                                          gidx_d[:, b0 // 16:(b0 + cols_b) // 16])
                        pss = []
                        for i, t in enumerate(ts):
                            ps = pp.tile([P, OUT], F32, tag=f"ps{i}")
                            nc.tensor.matmul(ps[:], lhsT=ident[:],
                                             rhs=h0t[:, t * OUT:(t + 1) * OUT],
                                             start=True, stop=False)
                            pss.append(ps)
                        msg = msgp.tile([P, maxbatch // P * OUT], BF16, tag="msg")
                        pi = 0
                        mcol = 0
                        for (q, o, n) in regions:
                            C = n // P
                            gb = gbp.tile([P, maxreg // P, OUT], F32, tag="gb")
                            while pi < len(pieces) and pieces[pi][1] < o + n:
                                (pq, po, pn) = pieces[pi]
                                assert pq == q and po >= o and po + pn <= o + n
                                c0 = (po - o) // P
                                nc.gpsimd.dma_gather(
                                    out_ap=gb[:, c0:c0 + pn // P, :],
                                    in_ap=table[q * QR:(q + 1) * QR, :],
                                    idxs_ap=ix[:, (po - b0) // 16:(po - b0 + pn) // 16],
                                    num_idxs=pn, num_idxs_reg=pn, elem_size=OUT)
                                pi += 1
                            nc.scalar.activation(
                                msg[:, mcol * OUT:(mcol + C) * OUT],
                                gb[:, :C, :].rearrange("p c d -> p (c d)"),
                                AF.Copy, bias=0.0, scale=1.0)
                            S = sp.tile([P, maxreg // P, P], BF16, tag="S")
                            nc.vector.tensor_tensor(
                                out=S[:, :C, :],
                                in0=gdl[:, o // P:o // P + C][:, :, None].to_broadcast([P, C, P]),
                                in1=iota[:, None, :].to_broadcast([P, C, P]),
                                op=ALU.is_equal)
                            nc.vector.tensor_tensor(
                                out=S[:, :C, :], in0=S[:, :C, :],
                                in1=gw[:, o // P:o // P + C][:, :, None].to_broadcast([P, C, P]),
                                op=ALU.mult)
                            # chunks of this region, in order
                            for j in range(C):
                                k_ = mcol + j
                                col, i = chunks[k_]
                                assert col == k_
                                nc.tensor.matmul(
                                    pss[i][:], lhsT=S[:, j, :],
                                    rhs=msg[:, k_ * OUT:(k_ + 1) * OUT],
                                    start=False, stop=(last_of_tile[i] == k_))
                            mcol += C
                        for i, t in enumerate(ts):
                            nc.vector.tensor_copy(hnew[:, t * OUT:(t + 1) * OUT],
                                                  pss[i][:])
                    if s < K - 1:
                        send_h()
                        all_gather()

            full = (NT - 1) * P
            nc.sync.dma_start(
                out_d.ap()[0:full, :].rearrange("(t p) d -> p t d", p=P),
                hnew[:, 0:(NT - 1) * OUT].rearrange("p (t d) -> p t d", d=OUT))
            nc.sync.dma_start(out_d[full:SH, :],
                              hnew[0:SH - full, (NT - 1) * OUT:NT * OUT])

    nc.compile()
    return nc


_CACHE = {}


def kernel(x, edge_index, W1, b1, W2, b2):
    x = np.asarray(x, np.float32)
    edge_index = np.asarray(edge_index)
    W1 = np.asarray(W1, np.float32)
    b1 = np.asarray(b1, np.float32)
    W2 = np.asarray(W2, np.float32)
    b2 = np.asarray(b2, np.float32)
    cfg = Cfg(N=x.shape[0], E=edge_index.shape[1], IN=x.shape[1],
              HID=W1.shape[1], OUT=W2.shape[1])
    sched, in_maps = preprocess(cfg, x, edge_index, W1, b1, W2, b2)
    key = ("k", cfg.N, cfg.E, sched["TOT"])
    if key not in _CACHE:
        _CACHE[key] = build(cfg, sched)
    nc = _CACHE[key]
    res = bass_utils.run_bass_kernel_spmd(nc, in_maps, core_ids=list(range(cfg.NC)))
    return np.concatenate([res.results[c]["out"] for c in range(cfg.NC)], axis=0)


if __name__ == "__main__":
    pass


# revision 11
# speedup vs baseline: 2.4325x; 2.4325x over previous
"""APPNP (GNN message passing) distributed Bass kernel for 8 TRN2 NeuronCores.

Strategy (graph/data parallel, "pull" form):
  - Nodes sharded by contiguous id range across 8 cores (12500 each).
  - MLP head computed on-device per shard (xT passed pre-transposed bf16).
  - Each propagation step: AllGather the full f32 h table (100000 x 64) into
    per-core DRAM, then each core gathers h[src] rows for its dst-range edges
    via dma_gather (int16 indices -> table split into 4 row-quarters), scales
    by edge norm via a weighted one-hot (DVE is_equal + mul), and scatter-adds
    into per-tile PSUM accumulators with TensorEngine matmuls
    (agg_tile += S'^T @ msg).  Teleport term enters PSUM as an extra
    identity-matmul with 0.1*h0 (bf16).
  - Edge structure (sort, cells, padding, norms) is preprocessed on host;
    slot layout is identical on all 8 cores (SPMD single graph).
"""

import sys

sys.path.insert(0, "/opt/trn_rl_repo")

import numpy as np
import concourse.bass as bass
import concourse.bacc as bacc
import concourse.tile as tile
import concourse.mybir as mybir
from concourse import bass_utils

F32 = mybir.dt.float32
BF16 = mybir.dt.bfloat16
I16 = mybir.dt.int16
I32 = mybir.dt.int32
AF = mybir.ActivationFunctionType
ALU = mybir.AluOpType

P = 128


class Cfg:
    def __init__(self, N=100000, E=3200000, IN=512, HID=256, OUT=64, K=10,
                 alpha=0.1, NC=8, NQ=4, TB=4):
        self.N, self.E, self.IN, self.HID, self.OUT, self.K = N, E, IN, HID, OUT, K
        self.alpha, self.NC, self.NQ = alpha, NC, NQ
        self.SH = N // NC                      # nodes per core
        self.NT = (self.SH + P - 1) // P       # dst tiles per core
        self.PSH = self.NT * P                 # padded shard rows
        self.QR = N // NQ                      # table rows per quarter (int16 range)
        assert N % NC == 0 and N % NQ == 0 and self.QR <= 32767
        self.TB = TB                           # tiles per batch
        self.NB = (self.NT + TB - 1) // TB     # batches
        self.CT = 2                            # tiles per gather call
        self.SCRATCH = 49152                   # swdge ring bytes (ring = /16 descs)
        self.NSQ = 4                           # swdge queues (round-robin gathers)
        self.KI = IN // P                      # k chunks of layer 1
        self.KH = HID // P                     # k chunks of layer 2
        self.RB = 512                          # mlp row batch
        assert IN % P == 0 and HID % P == 0 and OUT <= P


MAXG = 1024  # hw swdge ring: max descriptors per dma_gather call


def gather_pieces(cfg, cell, off):
    """Per-batch list of gather calls [(q, slot_off, n_slots)], each <= MAXG."""
    NT, NQ = cfg.NT, cfg.NQ
    out = []
    for b in range(cfg.NB):
        bts = list(range(b * cfg.TB, min((b + 1) * cfg.TB, NT)))
        pieces = []
        for g0 in range(0, len(bts), cfg.CT):
            gts = bts[g0:g0 + cfg.CT]
            for q in range(NQ):
                o = int(off[gts[0], q])
                n = int(cell[gts[0]:gts[-1] + 1, q].sum())
                p = o
                while p < o + n:
                    m = min(MAXG, o + n - p)
                    pieces.append((q, p, m))
                    p += m
        out.append(pieces)
    return out


def preprocess(cfg, x, edge_index, W1, b1, W2, b2):
    """Host-side graph preprocessing. Returns (static schedule, per-core in_maps)."""
    N, NC, SH, NT, NQ, QR = cfg.N, cfg.NC, cfg.SH, cfg.NT, cfg.NQ, cfg.QR
    src = np.concatenate([edge_index[0], np.arange(N, dtype=np.int64)])
    dst = np.concatenate([edge_index[1], np.arange(N, dtype=np.int64)])
    deg = np.bincount(dst, minlength=N).astype(np.float64)
    dinv = 1.0 / np.sqrt(deg)
    wall = ((1.0 - cfg.alpha) * dinv[src] * dinv[dst]).astype(np.float32)

    cores = []
    cnts = np.zeros((NC, NT, NQ), np.int64)
    for c in range(NC):
        sel = (dst >= c * SH) & (dst < (c + 1) * SH)
        s_c = src[sel]
        d_c = (dst[sel] - c * SH).astype(np.int64)
        w_c = wall[sel]
        t = d_c // P
        q = s_c // QR
        o = np.lexsort((d_c, q, t))
        s_c, d_c, w_c, t, q = s_c[o], d_c[o], w_c[o], t[o], q[o]
        cnts[c] = np.bincount(t * NQ + q, minlength=NT * NQ).reshape(NT, NQ)
        cores.append((s_c, d_c, w_c))

    # static per-(tile, quarter) slot counts: max over cores, rounded to 128
    cell = ((cnts.max(axis=0) + P - 1) // P * P).astype(np.int64)  # [NT, NQ]
    cell = np.maximum(cell, P)  # at least one chunk so matmul groups are non-empty
    TOT = int(cell.sum())

    # slot offsets in (batch -> quarter -> tile) order
    off = np.zeros((NT, NQ), np.int64)
    pos = 0
    for b in range(cfg.NB):
        bts = list(range(b * cfg.TB, min((b + 1) * cfg.TB, NT)))
        for g0 in range(0, len(bts), cfg.CT):
            gts = bts[g0:g0 + cfg.CT]
            for q in range(NQ):
                for t in gts:
                    off[t, q] = pos
                    pos += int(cell[t, q])
    assert pos == TOT

    in_maps = []
    for c in range(NC):
        s_c, d_c, w_c = cores[c]
        gidx = np.zeros(TOT, np.int16)
        gw = np.zeros(TOT, np.float32)
        gdl = np.zeros(TOT, np.float32)
        p0 = 0
        for t in range(NT):
            for q in range(NQ):
                n = int(cnts[c, t, q])
                o = off[t, q]
                gidx[o:o + n] = (s_c[p0:p0 + n] - q * QR).astype(np.int16)
                gw[o:o + n] = w_c[p0:p0 + n]
                gdl[o:o + n] = (d_c[p0:p0 + n] % P).astype(np.float32)
                p0 += n
        assert p0 == len(s_c)
        # idx wrap layout [16, TOT/16] per call, replicated to 128 partitions.
        # call = (batch, quarter) -> contiguous slot range
        gidx16 = np.zeros((16, TOT // 16), np.int16)
        for pieces in gather_pieces(cfg, cell, off):
            for (q, o, n) in pieces:
                blk = gidx[o:o + n].reshape(n // 16, 16).T  # j -> [j%16, j//16]
                gidx16[:, o // 16:(o + n) // 16] = blk
        gidx128 = np.tile(gidx16, (8, 1))  # replicate for 8 q7 cores
        # w / dstlocal in gather-out layout: slot j -> [j%128, j//128]
        gw128 = gw.reshape(TOT // P, P).T.astype(np.float32)
        gdl128 = gdl.reshape(TOT // P, P).T.astype(np.float32)
        bf16 = mybir.dt.np(mybir.dt.bfloat16)
        to_bf16 = lambda a: np.asarray(a, np.float32).astype(bf16)
        x_c = np.zeros((cfg.PSH, cfg.IN), np.float32)
        x_c[:SH] = x[c * SH:(c + 1) * SH]
        in_maps.append({
            "xt": to_bf16(np.ascontiguousarray(x_c.T)),
            "w1": to_bf16(W1),
            "b1": np.ascontiguousarray(b1.reshape(cfg.HID // P, P).T.astype(np.float32)),
            "w2": to_bf16(W2),
            "b2": to_bf16(b2.reshape(1, cfg.OUT)),
            "gidx": gidx128,
            "gw": to_bf16(gw128),
            "gdl": to_bf16(gdl128),
        })
    sched = {"cell": cell, "off": off, "TOT": TOT}
    return sched, in_maps


def build(cfg, sched):
    cell, off, TOT = sched["cell"], sched["off"], sched["TOT"]
    N, NT, NQ, QR, OUT, SH = cfg.N, cfg.NT, cfg.NQ, cfg.QR, cfg.OUT, cfg.SH
    NB, TB, K = cfg.NB, cfg.TB, cfg.K

    nc = bacc.Bacc("TRN2", target_bir_lowering=False, debug=False, num_devices=cfg.NC,
                   dynamic_dma_scratch_size=cfg.SCRATCH, num_swdge_queues=cfg.NSQ)

    xt_d = nc.declare_dram_parameter("xt", [cfg.IN, cfg.PSH], BF16, isOutput=False)
    w1_d = nc.declare_dram_parameter("w1", [cfg.IN, cfg.HID], BF16, isOutput=False)
    b1_d = nc.declare_dram_parameter("b1", [P, cfg.KH], F32, isOutput=False)
    w2_d = nc.declare_dram_parameter("w2", [cfg.HID, OUT], BF16, isOutput=False)
    b2_d = nc.declare_dram_parameter("b2", [1, OUT], BF16, isOutput=False)
    gidx_d = nc.declare_dram_parameter("gidx", [P, TOT // 16], I16, isOutput=False)
    gw_d = nc.declare_dram_parameter("gw", [P, TOT // P], BF16, isOutput=False)
    gdl_d = nc.declare_dram_parameter("gdl", [P, TOT // P], BF16, isOutput=False)
    out_d = nc.declare_dram_parameter("out", [SH, OUT], F32, isOutput=True)

    table = nc.dram_tensor("table", [N, OUT], F32, addr_space="Shared")
    send = nc.dram_tensor("send", [SH, OUT], F32)

    # per-batch call sizes (slots) and chunk schedule
    allpieces = gather_pieces(cfg, cell, off)
    batches = []
    for b in range(NB):
        ts = list(range(b * TB, min((b + 1) * TB, NT)))
        regions = []   # (q, slot_off, n): contiguous (group, quarter) spans for cast/S
        chunks = []
        col = 0
        for g0 in range(0, len(ts), cfg.CT):
            gts = ts[g0:g0 + cfg.CT]
            for q in range(NQ):
                n = int(cell[gts[0]:gts[-1] + 1, q].sum())
                regions.append((q, int(off[gts[0], q]), n))
                for t in gts:
                    for j in range(int(cell[t, q]) // P):
                        chunks.append((col, t - b * TB))
                        col += 1
        last_of_tile = {}
        for k_, (col_, i) in enumerate(chunks):
            last_of_tile[i] = k_
        batches.append((ts, allpieces[b], regions, chunks, last_of_tile))
    maxreg = max(n for (_, _, regions, _, _) in batches for (_, _, n) in regions)
    maxbatch = max(sum(n for (_, _, n) in regions) for (_, _, regions, _, _) in batches)

    with tile.TileContext(nc) as tc:
        with tc.tile_pool(name="const", bufs=1) as cp:
            iota = cp.tile([P, P], BF16, tag="iota")
            iota_i = cp.tile([P, P], I32, tag="iota_i")
            nc.gpsimd.iota(iota_i[:], pattern=[[1, P]], base=0, channel_multiplier=0)
            nc.vector.tensor_copy(iota[:], iota_i[:])
            iop = cp.tile([P, 1], I32, tag="iop")
            nc.gpsimd.iota(iop[:], pattern=[[0, 1]], base=0, channel_multiplier=1)
            iopb = cp.tile([P, 1], BF16, tag="iopb")
            nc.vector.tensor_copy(iopb[:], iop[:])
            ident = cp.tile([P, P], BF16, tag="ident")
            nc.vector.tensor_tensor(
                out=ident[:], in0=iopb[:].to_broadcast([P, P]), in1=iota[:],
                op=ALU.is_equal)
            ones = cp.tile([1, P], BF16, tag="ones")
            nc.gpsimd.memset(ones[:], 1.0)

            w1s = cp.tile([P, cfg.KI, cfg.HID], BF16, tag="w1s")
            nc.sync.dma_start(w1s[:], w1_d.ap().rearrange("(k p) h -> p k h", p=P))
            w2s = cp.tile([P, cfg.KH, OUT], BF16, tag="w2s")
            nc.sync.dma_start(w2s[:], w2_d.ap().rearrange("(k p) o -> p k o", p=P))
            b1s = cp.tile([P, cfg.KH], F32, tag="b1s")
            nc.sync.dma_start(b1s[:], b1_d[:, :])
            b2s = cp.tile([1, OUT], BF16, tag="b2s")
            nc.sync.dma_start(b2s[:], b2_d[:, :])
            gw = cp.tile([P, TOT // P], BF16, tag="gw")
            nc.sync.dma_start(gw[:], gw_d[:, :])
            gdl = cp.tile([P, TOT // P], BF16, tag="gdl")
            nc.sync.dma_start(gdl[:], gdl_d[:, :])

            h0t = cp.tile([P, NT * OUT], BF16, tag="h0t")    # 0.1 * h0, bf16
            hnew = cp.tile([P, NT * OUT], F32, tag="hnew")   # current h, f32

            # ---------------- MLP head ----------------
            with (
                tc.tile_pool(name="mxt", bufs=2 * cfg.KI) as mxt,
                tc.tile_pool(name="mp1", bufs=2, space="PSUM") as mp1,
                tc.tile_pool(name="mh1", bufs=2) as mh1,
                tc.tile_pool(name="mp2", bufs=2, space="PSUM") as mp2,
            ):
                RB = cfg.RB
                nrb = (cfg.PSH + RB - 1) // RB
                for b in range(nrb):
                    r0 = b * RB
                    rows = min(RB, cfg.PSH - r0)
                    xts = []
                    for k_ in range(cfg.KI):
                        xk = mxt.tile([P, RB], BF16, tag="xt")
                        nc.sync.dma_start(xk[:, :rows], xt_d[k_ * P:(k_ + 1) * P, r0:r0 + rows])
                        xts.append(xk)
                    h1t = mh1.tile([P, cfg.KH, RB], BF16, tag="h1t")
                    for fc in range(cfg.KH):
                        ps = mp1.tile([P, RB], F32, tag=f"mp1_{fc}")
                        for k_ in range(cfg.KI):
                            nc.tensor.matmul(
                                ps[:, :rows],
                                lhsT=w1s[:, k_, fc * P:(fc + 1) * P],
                                rhs=xts[k_][:, :rows],
                                start=(k_ == 0), stop=(k_ == cfg.KI - 1))
                        nc.scalar.activation(
                            h1t[:, fc, :rows], ps[:, :rows], AF.Relu,
                            bias=b1s[:, fc:fc + 1], scale=1.0)
                    for tl in range(rows // P):
                        gt = r0 // P + tl
                        p2 = mp2.tile([P, OUT], F32, tag="mp2")
                        for fc in range(cfg.KH):
                            nc.tensor.matmul(
                                p2[:], lhsT=h1t[:, fc, tl * P:(tl + 1) * P],
                                rhs=w2s[:, fc, :], start=(fc == 0), stop=False)
                        nc.tensor.matmul(p2[:], lhsT=ones[:1, :], rhs=b2s[:1, :],
                                         start=False, stop=True)
                        nc.scalar.activation(h0t[:, gt * OUT:(gt + 1) * OUT], p2[:],
                                             AF.Copy, bias=0.0, scale=cfg.alpha)
                        nc.vector.tensor_copy(hnew[:, gt * OUT:(gt + 1) * OUT], p2[:])

            def send_h():
                full = (NT - 1) * P
                nc.sync.dma_start(
                    send.ap()[0:full, :].rearrange("(t p) d -> p t d", p=P),
                    hnew[:, 0:(NT - 1) * OUT].rearrange("p (t d) -> p t d", d=OUT))
                rem = SH - full
                nc.sync.dma_start(
                    send[full:SH, :],
                    hnew[0:rem, (NT - 1) * OUT:NT * OUT])

            def all_gather():
                nc.gpsimd.collective_compute(
                    "AllGather", ALU.bypass,
                    ins=[send.ap().opt()], outs=[table.ap().opt()],
                    replica_groups=[list(range(cfg.NC))])

            send_h()
            all_gather()

            # ---------------- propagation ----------------
            with (
                tc.tile_pool(name="ixp", bufs=2) as ixp,
                tc.tile_pool(name="gbp", bufs=3) as gbp,
                tc.tile_pool(name="msgp", bufs=2) as msgp,
                tc.tile_pool(name="sp", bufs=3) as sp,
                tc.tile_pool(name="pp", bufs=2, space="PSUM") as pp,
            ):
                for s in range(K):
                    for (ts, pieces, regions, chunks, last_of_tile) in batches:
                        b0 = int(off[ts[0], 0])           # first slot of batch
                        cols_b = sum(n for (_, _, n) in regions)
                        ix = ixp.tile([P, maxbatch // 16], I16, tag="ix")
                        nc.sync.dma_start(ix[:, :cols_b // 16],
                                          gidx_d[:, b0 // 16:(b0 + cols_b) // 16])
                        pss = []
                        for i, t in enumerate(ts):
                            ps = pp.tile([P, OUT], F32, tag=f"ps{i}")
                            nc.tensor.matmul(ps[:], lhsT=ident[:],
                                             rhs=h0t[:, t * OUT:(t + 1) * OUT],
                                             start=True, stop=False)
                            pss.append(ps)
                        msg = msgp.tile([P, maxbatch // P * OUT], BF16, tag="msg")
                        pi = 0
                        mcol = 0
                        qrr = [0]
                        for (q, o, n) in regions:
                            C = n // P
                            gb = gbp.tile([P, maxreg // P, OUT], F32, tag="gb")
                            while pi < len(pieces) and pieces[pi][1] < o + n:
                                (pq, po, pn) = pieces[pi]
                                assert pq == q and po >= o and po + pn <= o + n
                                c0 = (po - o) // P
                                nc.gpsimd.dma_gather(
                                    out_ap=gb[:, c0:c0 + pn // P, :],
                                    in_ap=table[q * QR:(q + 1) * QR, :],
                                    idxs_ap=ix[:, (po - b0) // 16:(po - b0 + pn) // 16],
                                    num_idxs=pn, num_idxs_reg=pn, elem_size=OUT,
                                    queue_num=qrr[0] % cfg.NSQ)
                                qrr[0] += 1
                                pi += 1
                            nc.scalar.activation(
                                msg[:, mcol * OUT:(mcol + C) * OUT],
                                gb[:, :C, :].rearrange("p c d -> p (c d)"),
                                AF.Copy, bias=0.0, scale=1.0)
                            S = sp.tile([P, maxreg // P, P], BF16, tag="S")
                            nc.vector.tensor_tensor(
                                out=S[:, :C, :],
                                in0=gdl[:, o // P:o // P + C][:, :, None].to_broadcast([P, C, P]),
                                in1=iota[:, None, :].to_broadcast([P, C, P]),
                                op=ALU.is_equal)
                            nc.vector.tensor_tensor(
                                out=S[:, :C, :], in0=S[:, :C, :],
                                in1=gw[:, o // P:o // P + C][:, :, None].to_broadcast([P, C, P]),
                                op=ALU.mult)
                            # chunks of this region, in order
                            for j in range(C):
                                k_ = mcol + j
                                col, i = chunks[k_]
                                assert col == k_
                                nc.tensor.matmul(
                                    pss[i][:], lhsT=S[:, j, :],
                                    rhs=msg[:, k_ * OUT:(k_ + 1) * OUT],
                                    start=False, stop=(last_of_tile[i] == k_))
                            mcol += C
                        for i, t in enumerate(ts):
                            nc.vector.tensor_copy(hnew[:, t * OUT:(t + 1) * OUT],
                                                  pss[i][:])
                    if s < K - 1:
                        send_h()
                        all_gather()

            full = (NT - 1) * P
            nc.sync.dma_start(
                out_d.ap()[0:full, :].rearrange("(t p) d -> p t d", p=P),
                hnew[:, 0:(NT - 1) * OUT].rearrange("p (t d) -> p t d", d=OUT))
            nc.sync.dma_start(out_d[full:SH, :],
                              hnew[0:SH - full, (NT - 1) * OUT:NT * OUT])

    nc.compile()
    return nc


_CACHE = {}


def kernel(x, edge_index, W1, b1, W2, b2):
    x = np.asarray(x, np.float32)
    edge_index = np.asarray(edge_index)
    W1 = np.asarray(W1, np.float32)
    b1 = np.asarray(b1, np.float32)
    W2 = np.asarray(W2, np.float32)
    b2 = np.asarray(b2, np.float32)
    cfg = Cfg(N=x.shape[0], E=edge_index.shape[1], IN=x.shape[1],
              HID=W1.shape[1], OUT=W2.shape[1])
    sched, in_maps = preprocess(cfg, x, edge_index, W1, b1, W2, b2)
    key = ("k", cfg.N, cfg.E, sched["TOT"])
    if key not in _CACHE:
        _CACHE[key] = build(cfg, sched)
    nc = _CACHE[key]
    res = bass_utils.run_bass_kernel_spmd(nc, in_maps, core_ids=list(range(cfg.NC)))
    return np.concatenate([res.results[c]["out"] for c in range(cfg.NC)], axis=0)


if __name__ == "__main__":
    pass


# revision 12
# speedup vs baseline: 2.4528x; 1.0084x over previous
"""APPNP (GNN message passing) distributed Bass kernel for 8 TRN2 NeuronCores.

Strategy (graph/data parallel, "pull" form):
  - Nodes sharded by contiguous id range across 8 cores (12500 each).
  - MLP head computed on-device per shard (xT passed pre-transposed bf16).
  - Each propagation step: AllGather the full f32 h table (100000 x 64) into
    per-core DRAM, then each core gathers h[src] rows for its dst-range edges
    via dma_gather (int16 indices -> table split into 4 row-quarters), scales
    by edge norm via a weighted one-hot (DVE is_equal + mul), and scatter-adds
    into per-tile PSUM accumulators with TensorEngine matmuls
    (agg_tile += S'^T @ msg).  Teleport term enters PSUM as an extra
    identity-matmul with 0.1*h0 (bf16).
  - Edge structure (sort, cells, padding, norms) is preprocessed on host;
    slot layout is identical on all 8 cores (SPMD single graph).
"""

import sys

sys.path.insert(0, "/opt/trn_rl_repo")

import numpy as np
import concourse.bass as bass
import concourse.bacc as bacc
import concourse.tile as tile
import concourse.mybir as mybir
from concourse import bass_utils

F32 = mybir.dt.float32
BF16 = mybir.dt.bfloat16
I16 = mybir.dt.int16
I32 = mybir.dt.int32
AF = mybir.ActivationFunctionType
ALU = mybir.AluOpType

P = 128


class Cfg:
    def __init__(self, N=100000, E=3200000, IN=512, HID=256, OUT=64, K=10,
                 alpha=0.1, NC=8, NQ=4, TB=4):
        self.N, self.E, self.IN, self.HID, self.OUT, self.K = N, E, IN, HID, OUT, K
        self.alpha, self.NC, self.NQ = alpha, NC, NQ
        self.SH = N // NC                      # nodes per core
        self.NT = (self.SH + P - 1) // P       # dst tiles per core
        self.PSH = self.NT * P                 # padded shard rows
        self.QR = N // NQ                      # table rows per quarter (int16 range)
        assert N % NC == 0 and N % NQ == 0 and self.QR <= 32767
        self.TB = TB                           # tiles per batch
        self.NB = (self.NT + TB - 1) // TB     # batches
        self.CT = 2                            # tiles per gather call
        self.SCRATCH = 49152                   # swdge ring bytes (ring = /16 descs)
        self.NSQ = 4                           # swdge queues (round-robin gathers)
        self.KI = IN // P                      # k chunks of layer 1
        self.KH = HID // P                     # k chunks of layer 2
        self.RB = 512                          # mlp row batch
        assert IN % P == 0 and HID % P == 0 and OUT <= P


MAXG = 1024  # hw swdge ring: max descriptors per dma_gather call


def gather_pieces(cfg, cell, off):
    """Per-batch list of gather calls [(q, slot_off, n_slots)], each <= MAXG."""
    NT, NQ = cfg.NT, cfg.NQ
    out = []
    for b in range(cfg.NB):
        bts = list(range(b * cfg.TB, min((b + 1) * cfg.TB, NT)))
        pieces = []
        for g0 in range(0, len(bts), cfg.CT):
            gts = bts[g0:g0 + cfg.CT]
            for q in range(NQ):
                o = int(off[gts[0], q])
                n = int(cell[gts[0]:gts[-1] + 1, q].sum())
                p = o
                while p < o + n:
                    m = min(MAXG, o + n - p)
                    pieces.append((q, p, m))
                    p += m
        out.append(pieces)
    return out


def preprocess(cfg, x, edge_index, W1, b1, W2, b2):
    """Host-side graph preprocessing. Returns (static schedule, per-core in_maps)."""
    N, NC, SH, NT, NQ, QR = cfg.N, cfg.NC, cfg.SH, cfg.NT, cfg.NQ, cfg.QR
    src = np.concatenate([edge_index[0], np.arange(N, dtype=np.int64)])
    dst = np.concatenate([edge_index[1], np.arange(N, dtype=np.int64)])
    deg = np.bincount(dst, minlength=N).astype(np.float64)
    dinv = 1.0 / np.sqrt(deg)
    wall = ((1.0 - cfg.alpha) * dinv[src] * dinv[dst]).astype(np.float32)

    cores = []
    cnts = np.zeros((NC, NT, NQ), np.int64)
    for c in range(NC):
        sel = (dst >= c * SH) & (dst < (c + 1) * SH)
        s_c = src[sel]
        d_c = (dst[sel] - c * SH).astype(np.int64)
        w_c = wall[sel]
        t = d_c // P
        q = s_c // QR
        o = np.lexsort((d_c, q, t))
        s_c, d_c, w_c, t, q = s_c[o], d_c[o], w_c[o], t[o], q[o]
        cnts[c] = np.bincount(t * NQ + q, minlength=NT * NQ).reshape(NT, NQ)
        cores.append((s_c, d_c, w_c))

    # static per-(tile, quarter) slot counts: max over cores, rounded to 128
    cell = ((cnts.max(axis=0) + P - 1) // P * P).astype(np.int64)  # [NT, NQ]
    cell = np.maximum(cell, P)  # at least one chunk so matmul groups are non-empty
    TOT = int(cell.sum())

    # slot offsets in (batch -> quarter -> tile) order
    off = np.zeros((NT, NQ), np.int64)
    pos = 0
    for b in range(cfg.NB):
        bts = list(range(b * cfg.TB, min((b + 1) * cfg.TB, NT)))
        for g0 in range(0, len(bts), cfg.CT):
            gts = bts[g0:g0 + cfg.CT]
            for q in range(NQ):
                for t in gts:
                    off[t, q] = pos
                    pos += int(cell[t, q])
    assert pos == TOT

    in_maps = []
    for c in range(NC):
        s_c, d_c, w_c = cores[c]
        gidx = np.zeros(TOT, np.int16)
        gw = np.zeros(TOT, np.float32)
        gdl = np.zeros(TOT, np.float32)
        p0 = 0
        for t in range(NT):
            for q in range(NQ):
                n = int(cnts[c, t, q])
                o = off[t, q]
                gidx[o:o + n] = (s_c[p0:p0 + n] - q * QR).astype(np.int16)
                gw[o:o + n] = w_c[p0:p0 + n]
                gdl[o:o + n] = (d_c[p0:p0 + n] % P).astype(np.float32)
                p0 += n
        assert p0 == len(s_c)
        # idx wrap layout [16, TOT/16] per call, replicated to 128 partitions.
        # call = (batch, quarter) -> contiguous slot range
        gidx16 = np.zeros((16, TOT // 16), np.int16)
        for pieces in gather_pieces(cfg, cell, off):
            for (q, o, n) in pieces:
                blk = gidx[o:o + n].reshape(n // 16, 16).T  # j -> [j%16, j//16]
                gidx16[:, o // 16:(o + n) // 16] = blk
        gidx128 = np.tile(gidx16, (8, 1))  # replicate for 8 q7 cores
        bf16 = mybir.dt.np(mybir.dt.bfloat16)
        to_bf16 = lambda a: np.asarray(a, np.float32).astype(bf16)
        # streamed weighted one-hot: slot j=(c*128+p) -> sp[p, c*128 + dstloc_j] = w_j
        sl = np.arange(TOT)
        sp = np.zeros((TOT // P, P, P), bf16)
        sp[sl // P, sl % P, gdl.astype(np.int64)] = to_bf16(gw)
        sp128 = np.ascontiguousarray(sp.transpose(1, 0, 2).reshape(P, TOT))
        x_c = np.zeros((cfg.PSH, cfg.IN), np.float32)
        x_c[:SH] = x[c * SH:(c + 1) * SH]
        in_maps.append({
            "xt": to_bf16(np.ascontiguousarray(x_c.T)),
            "w1": to_bf16(W1),
            "b1": np.ascontiguousarray(b1.reshape(cfg.HID // P, P).T.astype(np.float32)),
            "w2": to_bf16(W2),
            "b2": to_bf16(b2.reshape(1, cfg.OUT)),
            "gidx": gidx128,
            "sp": sp128,
        })
    sched = {"cell": cell, "off": off, "TOT": TOT}
    return sched, in_maps


def build(cfg, sched):
    cell, off, TOT = sched["cell"], sched["off"], sched["TOT"]
    N, NT, NQ, QR, OUT, SH = cfg.N, cfg.NT, cfg.NQ, cfg.QR, cfg.OUT, cfg.SH
    NB, TB, K = cfg.NB, cfg.TB, cfg.K

    nc = bacc.Bacc("TRN2", target_bir_lowering=False, debug=False, num_devices=cfg.NC,
                   dynamic_dma_scratch_size=cfg.SCRATCH, num_swdge_queues=cfg.NSQ)

    xt_d = nc.declare_dram_parameter("xt", [cfg.IN, cfg.PSH], BF16, isOutput=False)
    w1_d = nc.declare_dram_parameter("w1", [cfg.IN, cfg.HID], BF16, isOutput=False)
    b1_d = nc.declare_dram_parameter("b1", [P, cfg.KH], F32, isOutput=False)
    w2_d = nc.declare_dram_parameter("w2", [cfg.HID, OUT], BF16, isOutput=False)
    b2_d = nc.declare_dram_parameter("b2", [1, OUT], BF16, isOutput=False)
    gidx_d = nc.declare_dram_parameter("gidx", [P, TOT // 16], I16, isOutput=False)
    sp_d = nc.declare_dram_parameter("sp", [P, TOT], BF16, isOutput=False)
    out_d = nc.declare_dram_parameter("out", [SH, OUT], F32, isOutput=True)

    table = nc.dram_tensor("table", [N, OUT], F32, addr_space="Shared")
    send = nc.dram_tensor("send", [SH, OUT], F32)

    # per-batch call sizes (slots) and chunk schedule
    allpieces = gather_pieces(cfg, cell, off)
    batches = []
    for b in range(NB):
        ts = list(range(b * TB, min((b + 1) * TB, NT)))
        regions = []   # (q, slot_off, n): contiguous (group, quarter) spans for cast/S
        chunks = []
        col = 0
        for g0 in range(0, len(ts), cfg.CT):
            gts = ts[g0:g0 + cfg.CT]
            for q in range(NQ):
                n = int(cell[gts[0]:gts[-1] + 1, q].sum())
                regions.append((q, int(off[gts[0], q]), n))
                for t in gts:
                    for j in range(int(cell[t, q]) // P):
                        chunks.append((col, t - b * TB))
                        col += 1
        last_of_tile = {}
        for k_, (col_, i) in enumerate(chunks):
            last_of_tile[i] = k_
        batches.append((ts, allpieces[b], regions, chunks, last_of_tile))
    maxreg = max(n for (_, _, regions, _, _) in batches for (_, _, n) in regions)
    maxbatch = max(sum(n for (_, _, n) in regions) for (_, _, regions, _, _) in batches)

    with tile.TileContext(nc) as tc:
        with tc.tile_pool(name="const", bufs=1) as cp:
            iota = cp.tile([P, P], BF16, tag="iota")
            iota_i = cp.tile([P, P], I32, tag="iota_i")
            nc.gpsimd.iota(iota_i[:], pattern=[[1, P]], base=0, channel_multiplier=0)
            nc.vector.tensor_copy(iota[:], iota_i[:])
            iop = cp.tile([P, 1], I32, tag="iop")
            nc.gpsimd.iota(iop[:], pattern=[[0, 1]], base=0, channel_multiplier=1)
            iopb = cp.tile([P, 1], BF16, tag="iopb")
            nc.vector.tensor_copy(iopb[:], iop[:])
            ident = cp.tile([P, P], BF16, tag="ident")
            nc.vector.tensor_tensor(
                out=ident[:], in0=iopb[:].to_broadcast([P, P]), in1=iota[:],
                op=ALU.is_equal)
            ones = cp.tile([1, P], BF16, tag="ones")
            nc.gpsimd.memset(ones[:], 1.0)

            w1s = cp.tile([P, cfg.KI, cfg.HID], BF16, tag="w1s")
            nc.sync.dma_start(w1s[:], w1_d.ap().rearrange("(k p) h -> p k h", p=P))
            w2s = cp.tile([P, cfg.KH, OUT], BF16, tag="w2s")
            nc.sync.dma_start(w2s[:], w2_d.ap().rearrange("(k p) o -> p k o", p=P))
            b1s = cp.tile([P, cfg.KH], F32, tag="b1s")
            nc.sync.dma_start(b1s[:], b1_d[:, :])
            b2s = cp.tile([1, OUT], BF16, tag="b2s")
            nc.sync.dma_start(b2s[:], b2_d[:, :])


            h0t = cp.tile([P, NT * OUT], BF16, tag="h0t")    # 0.1 * h0, bf16
            hnew = cp.tile([P, NT * OUT], F32, tag="hnew")   # current h, f32

            # ---------------- MLP head ----------------
            with (
                tc.tile_pool(name="mxt", bufs=2 * cfg.KI) as mxt,
                tc.tile_pool(name="mp1", bufs=2, space="PSUM") as mp1,
                tc.tile_pool(name="mh1", bufs=2) as mh1,
                tc.tile_pool(name="mp2", bufs=2, space="PSUM") as mp2,
            ):
                RB = cfg.RB
                nrb = (cfg.PSH + RB - 1) // RB
                for b in range(nrb):
                    r0 = b * RB
                    rows = min(RB, cfg.PSH - r0)
                    xts = []
                    for k_ in range(cfg.KI):
                        xk = mxt.tile([P, RB], BF16, tag="xt")
                        nc.sync.dma_start(xk[:, :rows], xt_d[k_ * P:(k_ + 1) * P, r0:r0 + rows])
                        xts.append(xk)
                    h1t = mh1.tile([P, cfg.KH, RB], BF16, tag="h1t")
                    for fc in range(cfg.KH):
                        ps = mp1.tile([P, RB], F32, tag=f"mp1_{fc}")
                        for k_ in range(cfg.KI):
                            nc.tensor.matmul(
                                ps[:, :rows],
                                lhsT=w1s[:, k_, fc * P:(fc + 1) * P],
                                rhs=xts[k_][:, :rows],
                                start=(k_ == 0), stop=(k_ == cfg.KI - 1))
                        nc.scalar.activation(
                            h1t[:, fc, :rows], ps[:, :rows], AF.Relu,
                            bias=b1s[:, fc:fc + 1], scale=1.0)
                    for tl in range(rows // P):
                        gt = r0 // P + tl
                        p2 = mp2.tile([P, OUT], F32, tag="mp2")
                        for fc in range(cfg.KH):
                            nc.tensor.matmul(
                                p2[:], lhsT=h1t[:, fc, tl * P:(tl + 1) * P],
                                rhs=w2s[:, fc, :], start=(fc == 0), stop=False)
                        nc.tensor.matmul(p2[:], lhsT=ones[:1, :], rhs=b2s[:1, :],
                                         start=False, stop=True)
                        nc.scalar.activation(h0t[:, gt * OUT:(gt + 1) * OUT], p2[:],
                                             AF.Copy, bias=0.0, scale=cfg.alpha)
                        nc.vector.tensor_copy(hnew[:, gt * OUT:(gt + 1) * OUT], p2[:])

            def send_h():
                full = (NT - 1) * P
                nc.sync.dma_start(
                    send.ap()[0:full, :].rearrange("(t p) d -> p t d", p=P),
                    hnew[:, 0:(NT - 1) * OUT].rearrange("p (t d) -> p t d", d=OUT))
                rem = SH - full
                nc.sync.dma_start(
                    send[full:SH, :],
                    hnew[0:rem, (NT - 1) * OUT:NT * OUT])

            def all_gather():
                nc.gpsimd.collective_compute(
                    "AllGather", ALU.bypass,
                    ins=[send.ap().opt()], outs=[table.ap().opt()],
                    replica_groups=[list(range(cfg.NC))])

            send_h()
            all_gather()

            # ---------------- propagation ----------------
            with (
                tc.tile_pool(name="ixp", bufs=2) as ixp,
                tc.tile_pool(name="gbp", bufs=3) as gbp,
                tc.tile_pool(name="msgp", bufs=2) as msgp,
                tc.tile_pool(name="spp", bufs=3) as spp,
                tc.tile_pool(name="pp", bufs=2, space="PSUM") as pp,
            ):
                for s in range(K):
                    for (ts, pieces, regions, chunks, last_of_tile) in batches:
                        b0 = int(off[ts[0], 0])           # first slot of batch
                        cols_b = sum(n for (_, _, n) in regions)
                        ix = ixp.tile([P, maxbatch // 16], I16, tag="ix")
                        nc.sync.dma_start(ix[:, :cols_b // 16],
                                          gidx_d[:, b0 // 16:(b0 + cols_b) // 16])
                        pss = []
                        for i, t in enumerate(ts):
                            ps = pp.tile([P, OUT], F32, tag=f"ps{i}")
                            nc.tensor.matmul(ps[:], lhsT=ident[:],
                                             rhs=h0t[:, t * OUT:(t + 1) * OUT],
                                             start=True, stop=False)
                            pss.append(ps)
                        msg = msgp.tile([P, maxbatch // P * OUT], BF16, tag="msg")
                        pi = 0
                        mcol = 0
                        qrr = [0]
                        for (q, o, n) in regions:
                            C = n // P
                            gb = gbp.tile([P, maxreg // P, OUT], F32, tag="gb")
                            while pi < len(pieces) and pieces[pi][1] < o + n:
                                (pq, po, pn) = pieces[pi]
                                assert pq == q and po >= o and po + pn <= o + n
                                c0 = (po - o) // P
                                nc.gpsimd.dma_gather(
                                    out_ap=gb[:, c0:c0 + pn // P, :],
                                    in_ap=table[q * QR:(q + 1) * QR, :],
                                    idxs_ap=ix[:, (po - b0) // 16:(po - b0 + pn) // 16],
                                    num_idxs=pn, num_idxs_reg=pn, elem_size=OUT,
                                    queue_num=qrr[0] % cfg.NSQ)
                                qrr[0] += 1
                                pi += 1
                            nc.scalar.activation(
                                msg[:, mcol * OUT:(mcol + C) * OUT],
                                gb[:, :C, :].rearrange("p c d -> p (c d)"),
                                AF.Copy, bias=0.0, scale=1.0)
                            S = spp.tile([P, maxreg // P, P], BF16, tag="S")
                            nc.sync.dma_start(
                                S[:, :C, :].rearrange("p c d -> p (c d)"),
                                sp_d[:, o:o + n])
                            # chunks of this region, in order
                            for j in range(C):
                                k_ = mcol + j
                                col, i = chunks[k_]
                                assert col == k_
                                nc.tensor.matmul(
                                    pss[i][:], lhsT=S[:, j, :],
                                    rhs=msg[:, k_ * OUT:(k_ + 1) * OUT],
                                    start=False, stop=(last_of_tile[i] == k_))
                            mcol += C
                        for i, t in enumerate(ts):
                            nc.vector.tensor_copy(hnew[:, t * OUT:(t + 1) * OUT],
                                                  pss[i][:])
                    if s < K - 1:
                        send_h()
                        all_gather()

            full = (NT - 1) * P
            nc.sync.dma_start(
                out_d.ap()[0:full, :].rearrange("(t p) d -> p t d", p=P),
                hnew[:, 0:(NT - 1) * OUT].rearrange("p (t d) -> p t d", d=OUT))
            nc.sync.dma_start(out_d[full:SH, :],
                              hnew[0:SH - full, (NT - 1) * OUT:NT * OUT])

    nc.compile()
    return nc


_CACHE = {}


def kernel(x, edge_index, W1, b1, W2, b2):
    x = np.asarray(x, np.float32)
    edge_index = np.asarray(edge_index)
    W1 = np.asarray(W1, np.float32)
    b1 = np.asarray(b1, np.float32)
    W2 = np.asarray(W2, np.float32)
    b2 = np.asarray(b2, np.float32)
    cfg = Cfg(N=x.shape[0], E=edge_index.shape[1], IN=x.shape[1],
              HID=W1.shape[1], OUT=W2.shape[1])
    sched, in_maps = preprocess(cfg, x, edge_index, W1, b1, W2, b2)
    key = ("k", cfg.N, cfg.E, sched["TOT"])
    if key not in _CACHE:
        _CACHE[key] = build(cfg, sched)
    nc = _CACHE[key]
    res = bass_utils.run_bass_kernel_spmd(nc, in_maps, core_ids=list(range(cfg.NC)))
    return np.concatenate([res.results[c]["out"] for c in range(cfg.NC)], axis=0)


if __name__ == "__main__":
    pass


# revision 13
# speedup vs baseline: 2.6606x; 1.0847x over previous
"""APPNP (GNN message passing) distributed Bass kernel for 8 TRN2 NeuronCores.

Strategy (graph/data parallel, "pull" form):
  - Nodes sharded by contiguous id range across 8 cores (12500 each).
  - MLP head computed on-device per shard (xT passed pre-transposed bf16).
  - Each propagation step: AllGather the full f32 h table (100000 x 64) into
    per-core DRAM, then each core gathers h[src] rows for its dst-range edges
    via dma_gather (int16 indices -> table split into 4 row-quarters), scales
    by edge norm via a weighted one-hot (DVE is_equal + mul), and scatter-adds
    into per-tile PSUM accumulators with TensorEngine matmuls
    (agg_tile += S'^T @ msg).  Teleport term enters PSUM as an extra
    identity-matmul with 0.1*h0 (bf16).
  - Edge structure (sort, cells, padding, norms) is preprocessed on host;
    slot layout is identical on all 8 cores (SPMD single graph).
"""

import sys

sys.path.insert(0, "/opt/trn_rl_repo")

import numpy as np
import concourse.bass as bass
import concourse.bacc as bacc
import concourse.tile as tile
import concourse.mybir as mybir
from concourse import bass_utils

F32 = mybir.dt.float32
BF16 = mybir.dt.bfloat16
I16 = mybir.dt.int16
I32 = mybir.dt.int32
AF = mybir.ActivationFunctionType
ALU = mybir.AluOpType

P = 128


class Cfg:
    def __init__(self, N=100000, E=3200000, IN=512, HID=256, OUT=64, K=10,
                 alpha=0.1, NC=8, NQ=4, TB=4):
        self.N, self.E, self.IN, self.HID, self.OUT, self.K = N, E, IN, HID, OUT, K
        self.alpha, self.NC, self.NQ = alpha, NC, NQ
        self.SH = N // NC                      # nodes per core
        self.NT = (self.SH + P - 1) // P       # dst tiles per core
        self.PSH = self.NT * P                 # padded shard rows
        self.QR = N // NQ                      # table rows per quarter (int16 range)
        assert N % NC == 0 and N % NQ == 0 and self.QR <= 32767
        self.TB = TB                           # tiles per batch
        self.NB = (self.NT + TB - 1) // TB     # batches
        self.CT = 2                            # tiles per gather call
        self.SCRATCH = 49152                   # swdge ring bytes (ring = /16 descs)
        self.NSQ = 4                           # swdge queues (round-robin gathers)
        self.KI = IN // P                      # k chunks of layer 1
        self.KH = HID // P                     # k chunks of layer 2
        self.RB = 512                          # mlp row batch
        assert IN % P == 0 and HID % P == 0 and OUT <= P


MAXG = 1024  # hw swdge ring: max descriptors per dma_gather call


def gather_pieces(cfg, cell, off):
    """Per-batch list of gather calls [(q, slot_off, n_slots)], each <= MAXG."""
    NT, NQ = cfg.NT, cfg.NQ
    out = []
    for b in range(cfg.NB):
        bts = list(range(b * cfg.TB, min((b + 1) * cfg.TB, NT)))
        pieces = []
        for g0 in range(0, len(bts), cfg.CT):
            gts = bts[g0:g0 + cfg.CT]
            for q in range(NQ):
                o = int(off[gts[0], q])
                n = int(cell[gts[0]:gts[-1] + 1, q].sum())
                p = o
                while p < o + n:
                    m = min(MAXG, o + n - p)
                    pieces.append((q, p, m))
                    p += m
        out.append(pieces)
    return out


def preprocess(cfg, x, edge_index, W1, b1, W2, b2):
    """Host-side graph preprocessing. Returns (static schedule, per-core in_maps)."""
    N, NC, SH, NT, NQ, QR = cfg.N, cfg.NC, cfg.SH, cfg.NT, cfg.NQ, cfg.QR
    src = np.concatenate([edge_index[0], np.arange(N, dtype=np.int64)])
    dst = np.concatenate([edge_index[1], np.arange(N, dtype=np.int64)])
    deg = np.bincount(dst, minlength=N).astype(np.float64)
    dinv = 1.0 / np.sqrt(deg)
    wall = ((1.0 - cfg.alpha) * dinv[src] * dinv[dst]).astype(np.float32)

    cores = []
    cnts = np.zeros((NC, NT, NQ), np.int64)
    for c in range(NC):
        sel = (dst >= c * SH) & (dst < (c + 1) * SH)
        s_c = src[sel]
        d_c = (dst[sel] - c * SH).astype(np.int64)
        w_c = wall[sel]
        t = d_c // P
        q = s_c // QR
        o = np.lexsort((d_c, q, t))
        s_c, d_c, w_c, t, q = s_c[o], d_c[o], w_c[o], t[o], q[o]
        cnts[c] = np.bincount(t * NQ + q, minlength=NT * NQ).reshape(NT, NQ)
        cores.append((s_c, d_c, w_c))

    # static per-(tile, quarter) slot counts: max over cores, rounded to 128
    cell = ((cnts.max(axis=0) + P - 1) // P * P).astype(np.int64)  # [NT, NQ]
    cell = np.maximum(cell, P)  # at least one chunk so matmul groups are non-empty
    TOT = int(cell.sum())

    # slot offsets in (batch -> quarter -> tile) order
    off = np.zeros((NT, NQ), np.int64)
    pos = 0
    for b in range(cfg.NB):
        bts = list(range(b * cfg.TB, min((b + 1) * cfg.TB, NT)))
        for g0 in range(0, len(bts), cfg.CT):
            gts = bts[g0:g0 + cfg.CT]
            for q in range(NQ):
                for t in gts:
                    off[t, q] = pos
                    pos += int(cell[t, q])
    assert pos == TOT

    in_maps = []
    for c in range(NC):
        s_c, d_c, w_c = cores[c]
        gidx = np.zeros(TOT, np.int16)
        gw = np.zeros(TOT, np.float32)
        gdl = np.zeros(TOT, np.float32)
        p0 = 0
        for t in range(NT):
            for q in range(NQ):
                n = int(cnts[c, t, q])
                o = off[t, q]
                gidx[o:o + n] = (s_c[p0:p0 + n] - q * QR).astype(np.int16)
                gw[o:o + n] = w_c[p0:p0 + n]
                gdl[o:o + n] = (d_c[p0:p0 + n] % P).astype(np.float32)
                p0 += n
        assert p0 == len(s_c)
        # idx wrap layout [16, TOT/16] per call, replicated to 128 partitions.
        # call = (batch, quarter) -> contiguous slot range
        gidx16 = np.zeros((16, TOT // 16), np.int16)
        for pieces in gather_pieces(cfg, cell, off):
            for (q, o, n) in pieces:
                blk = gidx[o:o + n].reshape(n // 16, 16).T  # j -> [j%16, j//16]
                gidx16[:, o // 16:(o + n) // 16] = blk
        gidx128 = np.tile(gidx16, (8, 1))  # replicate for 8 q7 cores
        bf16 = mybir.dt.np(mybir.dt.bfloat16)
        to_bf16 = lambda a: np.asarray(a, np.float32).astype(bf16)
        # streamed weighted one-hot: slot j=(c*128+p) -> sp[p, c*128 + dstloc_j] = w_j
        sl = np.arange(TOT)
        sp = np.zeros((TOT // P, P, P), bf16)
        sp[sl // P, sl % P, gdl.astype(np.int64)] = to_bf16(gw)
        sp128 = np.ascontiguousarray(sp.transpose(1, 0, 2).reshape(P, TOT))
        x_c = np.zeros((cfg.PSH, cfg.IN), np.float32)
        x_c[:SH] = x[c * SH:(c + 1) * SH]
        in_maps.append({
            "xt": to_bf16(np.ascontiguousarray(x_c.T)),
            "w1": to_bf16(W1),
            "b1": np.ascontiguousarray(b1.reshape(cfg.HID // P, P).T.astype(np.float32)),
            "w2": to_bf16(W2),
            "b2": to_bf16(b2.reshape(1, cfg.OUT)),
            "gidx": gidx128,
            "sp": sp128,
        })
    sched = {"cell": cell, "off": off, "TOT": TOT}
    return sched, in_maps


def build(cfg, sched):
    cell, off, TOT = sched["cell"], sched["off"], sched["TOT"]
    N, NT, NQ, QR, OUT, SH = cfg.N, cfg.NT, cfg.NQ, cfg.QR, cfg.OUT, cfg.SH
    NB, TB, K = cfg.NB, cfg.TB, cfg.K

    nc = bacc.Bacc("TRN2", target_bir_lowering=False, debug=False, num_devices=cfg.NC,
                   dynamic_dma_scratch_size=cfg.SCRATCH, num_swdge_queues=cfg.NSQ)

    xt_d = nc.declare_dram_parameter("xt", [cfg.IN, cfg.PSH], BF16, isOutput=False)
    w1_d = nc.declare_dram_parameter("w1", [cfg.IN, cfg.HID], BF16, isOutput=False)
    b1_d = nc.declare_dram_parameter("b1", [P, cfg.KH], F32, isOutput=False)
    w2_d = nc.declare_dram_parameter("w2", [cfg.HID, OUT], BF16, isOutput=False)
    b2_d = nc.declare_dram_parameter("b2", [1, OUT], BF16, isOutput=False)
    gidx_d = nc.declare_dram_parameter("gidx", [P, TOT // 16], I16, isOutput=False)
    sp_d = nc.declare_dram_parameter("sp", [P, TOT], BF16, isOutput=False)
    out_d = nc.declare_dram_parameter("out", [SH, OUT], F32, isOutput=True)

    table = nc.dram_tensor("table", [N, OUT], F32, addr_space="Shared")
    send = nc.dram_tensor("send", [SH, OUT], F32)

    # per-batch call sizes (slots) and chunk schedule
    allpieces = gather_pieces(cfg, cell, off)
    batches = []
    for b in range(NB):
        ts = list(range(b * TB, min((b + 1) * TB, NT)))
        regions = []   # (q, slot_off, n): contiguous (group, quarter) spans for cast/S
        chunks = []
        col = 0
        for g0 in range(0, len(ts), cfg.CT):
            gts = ts[g0:g0 + cfg.CT]
            for q in range(NQ):
                n = int(cell[gts[0]:gts[-1] + 1, q].sum())
                regions.append((q, int(off[gts[0], q]), n))
                for t in gts:
                    for j in range(int(cell[t, q]) // P):
                        chunks.append((col, t - b * TB))
                        col += 1
        last_of_tile = {}
        for k_, (col_, i) in enumerate(chunks):
            last_of_tile[i] = k_
        batches.append((ts, allpieces[b], regions, chunks, last_of_tile))
    maxreg = max(n for (_, _, regions, _, _) in batches for (_, _, n) in regions)
    maxbatch = max(sum(n for (_, _, n) in regions) for (_, _, regions, _, _) in batches)

    with tile.TileContext(nc) as tc:
        with tc.tile_pool(name="const", bufs=1) as cp:
            iota = cp.tile([P, P], BF16, tag="iota")
            iota_i = cp.tile([P, P], I32, tag="iota_i")
            nc.gpsimd.iota(iota_i[:], pattern=[[1, P]], base=0, channel_multiplier=0)
            nc.vector.tensor_copy(iota[:], iota_i[:])
            iop = cp.tile([P, 1], I32, tag="iop")
            nc.gpsimd.iota(iop[:], pattern=[[0, 1]], base=0, channel_multiplier=1)
            iopb = cp.tile([P, 1], BF16, tag="iopb")
            nc.vector.tensor_copy(iopb[:], iop[:])
            ident = cp.tile([P, P], BF16, tag="ident")
            nc.vector.tensor_tensor(
                out=ident[:], in0=iopb[:].to_broadcast([P, P]), in1=iota[:],
                op=ALU.is_equal)
            ones = cp.tile([1, P], BF16, tag="ones")
            nc.gpsimd.memset(ones[:], 1.0)

            w1s = cp.tile([P, cfg.KI, cfg.HID], BF16, tag="w1s")
            nc.sync.dma_start(w1s[:], w1_d.ap().rearrange("(k p) h -> p k h", p=P))
            w2s = cp.tile([P, cfg.KH, OUT], BF16, tag="w2s")
            nc.sync.dma_start(w2s[:], w2_d.ap().rearrange("(k p) o -> p k o", p=P))
            b1s = cp.tile([P, cfg.KH], F32, tag="b1s")
            nc.sync.dma_start(b1s[:], b1_d[:, :])
            b2s = cp.tile([1, OUT], BF16, tag="b2s")
            nc.sync.dma_start(b2s[:], b2_d[:, :])


            h0t = cp.tile([P, NT * OUT], BF16, tag="h0t")    # 0.1 * h0, bf16
            hnew = cp.tile([P, NT * OUT], F32, tag="hnew")   # current h, f32

            # ---------------- MLP head ----------------
            with (
                tc.tile_pool(name="mxt", bufs=2 * cfg.KI) as mxt,
                tc.tile_pool(name="mp1", bufs=2, space="PSUM") as mp1,
                tc.tile_pool(name="mh1", bufs=2) as mh1,
                tc.tile_pool(name="mp2", bufs=2, space="PSUM") as mp2,
            ):
                RB = cfg.RB
                nrb = (cfg.PSH + RB - 1) // RB
                for b in range(nrb):
                    r0 = b * RB
                    rows = min(RB, cfg.PSH - r0)
                    xts = []
                    for k_ in range(cfg.KI):
                        xk = mxt.tile([P, RB], BF16, tag="xt")
                        nc.sync.dma_start(xk[:, :rows], xt_d[k_ * P:(k_ + 1) * P, r0:r0 + rows])
                        xts.append(xk)
                    h1t = mh1.tile([P, cfg.KH, RB], BF16, tag="h1t")
                    for fc in range(cfg.KH):
                        ps = mp1.tile([P, RB], F32, tag=f"mp1_{fc}")
                        for k_ in range(cfg.KI):
                            nc.tensor.matmul(
                                ps[:, :rows],
                                lhsT=w1s[:, k_, fc * P:(fc + 1) * P],
                                rhs=xts[k_][:, :rows],
                                start=(k_ == 0), stop=(k_ == cfg.KI - 1))
                        nc.scalar.activation(
                            h1t[:, fc, :rows], ps[:, :rows], AF.Relu,
                            bias=b1s[:, fc:fc + 1], scale=1.0)
                    for tl in range(rows // P):
                        gt = r0 // P + tl
                        p2 = mp2.tile([P, OUT], F32, tag="mp2")
                        for fc in range(cfg.KH):
                            nc.tensor.matmul(
                                p2[:], lhsT=h1t[:, fc, tl * P:(tl + 1) * P],
                                rhs=w2s[:, fc, :], start=(fc == 0), stop=False)
                        nc.tensor.matmul(p2[:], lhsT=ones[:1, :], rhs=b2s[:1, :],
                                         start=False, stop=True)
                        nc.scalar.activation(h0t[:, gt * OUT:(gt + 1) * OUT], p2[:],
                                             AF.Copy, bias=0.0, scale=cfg.alpha)
                        nc.vector.tensor_copy(hnew[:, gt * OUT:(gt + 1) * OUT], p2[:])

            def send_h():
                full = (NT - 1) * P
                nc.sync.dma_start(
                    send.ap()[0:full, :].rearrange("(t p) d -> p t d", p=P),
                    hnew[:, 0:(NT - 1) * OUT].rearrange("p (t d) -> p t d", d=OUT))
                rem = SH - full
                nc.sync.dma_start(
                    send[full:SH, :],
                    hnew[0:rem, (NT - 1) * OUT:NT * OUT])

            def all_gather():
                nc.gpsimd.collective_compute(
                    "AllGather", ALU.bypass,
                    ins=[send.ap().opt()], outs=[table.ap().opt()],
                    replica_groups=[list(range(cfg.NC))])

            send_h()
            all_gather()

            # ---------------- propagation ----------------
            with (
                tc.tile_pool(name="ixp", bufs=3) as ixp,
                tc.tile_pool(name="gbp", bufs=8) as gbp,
                tc.tile_pool(name="msgp", bufs=3) as msgp,
                tc.tile_pool(name="spp", bufs=4) as spp,
                tc.tile_pool(name="pp", bufs=2, space="PSUM") as pp,
            ):
                for s in range(K):
                    for (ts, pieces, regions, chunks, last_of_tile) in batches:
                        b0 = int(off[ts[0], 0])           # first slot of batch
                        cols_b = sum(n for (_, _, n) in regions)
                        ix = ixp.tile([P, maxbatch // 16], I16, tag="ix")
                        nc.sync.dma_start(ix[:, :cols_b // 16],
                                          gidx_d[:, b0 // 16:(b0 + cols_b) // 16])
                        pss = []
                        for i, t in enumerate(ts):
                            ps = pp.tile([P, OUT], F32, tag=f"ps{i}")
                            nc.tensor.matmul(ps[:], lhsT=ident[:],
                                             rhs=h0t[:, t * OUT:(t + 1) * OUT],
                                             start=True, stop=False)
                            pss.append(ps)
                        msg = msgp.tile([P, maxbatch // P * OUT], BF16, tag="msg")
                        pi = 0
                        mcol = 0
                        qrr = [0]
                        for (q, o, n) in regions:
                            C = n // P
                            gb = gbp.tile([P, maxreg // P, OUT], F32, tag="gb")
                            while pi < len(pieces) and pieces[pi][1] < o + n:
                                (pq, po, pn) = pieces[pi]
                                assert pq == q and po >= o and po + pn <= o + n
                                c0 = (po - o) // P
                                nc.gpsimd.dma_gather(
                                    out_ap=gb[:, c0:c0 + pn // P, :],
                                    in_ap=table[q * QR:(q + 1) * QR, :],
                                    idxs_ap=ix[:, (po - b0) // 16:(po - b0 + pn) // 16],
                                    num_idxs=pn, num_idxs_reg=pn, elem_size=OUT,
                                    queue_num=qrr[0] % cfg.NSQ)
                                qrr[0] += 1
                                pi += 1
                            nc.scalar.activation(
                                msg[:, mcol * OUT:(mcol + C) * OUT],
                                gb[:, :C, :].rearrange("p c d -> p (c d)"),
                                AF.Copy, bias=0.0, scale=1.0)
                            S = spp.tile([P, maxreg // P, P], BF16, tag="S")
                            nc.sync.dma_start(
                                S[:, :C, :].rearrange("p c d -> p (c d)"),
                                sp_d[:, o:o + n])
                            # chunks of this region, in order
                            for j in range(C):
                                k_ = mcol + j
                                col, i = chunks[k_]
                                assert col == k_
                                nc.tensor.matmul(
                                    pss[i][:], lhsT=S[:, j, :],
                                    rhs=msg[:, k_ * OUT:(k_ + 1) * OUT],
                                    start=False, stop=(last_of_tile[i] == k_))
                            mcol += C
                        for i, t in enumerate(ts):
                            nc.vector.tensor_copy(hnew[:, t * OUT:(t + 1) * OUT],
                                                  pss[i][:])
                    if s < K - 1:
                        send_h()
                        all_gather()

            full = (NT - 1) * P
            nc.sync.dma_start(
                out_d.ap()[0:full, :].rearrange("(t p) d -> p t d", p=P),
                hnew[:, 0:(NT - 1) * OUT].rearrange("p (t d) -> p t d", d=OUT))
            nc.sync.dma_start(out_d[full:SH, :],
                              hnew[0:SH - full, (NT - 1) * OUT:NT * OUT])

    nc.compile()
    return nc


_CACHE = {}


def kernel(x, edge_index, W1, b1, W2, b2):
    x = np.asarray(x, np.float32)
    edge_index = np.asarray(edge_index)
    W1 = np.asarray(W1, np.float32)
    b1 = np.asarray(b1, np.float32)
    W2 = np.asarray(W2, np.float32)
    b2 = np.asarray(b2, np.float32)
    cfg = Cfg(N=x.shape[0], E=edge_index.shape[1], IN=x.shape[1],
              HID=W1.shape[1], OUT=W2.shape[1])
    sched, in_maps = preprocess(cfg, x, edge_index, W1, b1, W2, b2)
    key = ("k", cfg.N, cfg.E, sched["TOT"])
    if key not in _CACHE:
        _CACHE[key] = build(cfg, sched)
    nc = _CACHE[key]
    res = bass_utils.run_bass_kernel_spmd(nc, in_maps, core_ids=list(range(cfg.NC)))
    return np.concatenate([res.results[c]["out"] for c in range(cfg.NC)], axis=0)


if __name__ == "__main__":
    pass


# revision 17
# speedup vs baseline: 3.0755x; 1.1560x over previous
"""APPNP (GNN message passing) distributed Bass kernel for 8 TRN2 NeuronCores.

Strategy (graph/data parallel, "pull" form):
  - Nodes sharded by contiguous id range across 8 cores (12500 each).
  - MLP head computed on-device per shard (xT passed pre-transposed bf16).
  - Each propagation step: AllGather the full f32 h table (100000 x 64) into
    per-core DRAM, then each core gathers h[src] rows for its dst-range edges
    via dma_gather (int16 indices -> table split into 4 row-quarters), scales
    by edge norm via a weighted one-hot (DVE is_equal + mul), and scatter-adds
    into per-tile PSUM accumulators with TensorEngine matmuls
    (agg_tile += S'^T @ msg).  Teleport term enters PSUM as an extra
    identity-matmul with 0.1*h0 (bf16).
  - Edge structure (sort, cells, padding, norms) is preprocessed on host;
    slot layout is identical on all 8 cores (SPMD single graph).
"""

import sys

sys.path.insert(0, "/opt/trn_rl_repo")

import numpy as np
import concourse.bass as bass
import concourse.bacc as bacc
import concourse.tile as tile
import concourse.mybir as mybir
from concourse import bass_utils

F32 = mybir.dt.float32
BF16 = mybir.dt.bfloat16
I16 = mybir.dt.int16
I32 = mybir.dt.int32
AF = mybir.ActivationFunctionType
ALU = mybir.AluOpType

P = 128


class Cfg:
    def __init__(self, N=100000, E=3200000, IN=512, HID=256, OUT=64, K=10,
                 alpha=0.1, NC=8, NQ=5, TB=4, pad_idx=-1):
        self.N, self.E, self.IN, self.HID, self.OUT, self.K = N, E, IN, HID, OUT, K
        self.alpha, self.NC, self.NQ = alpha, NC, NQ
        self.SH = N // NC                      # nodes per core
        self.NT = (self.SH + P - 1) // P       # dst tiles per core
        self.PSH = self.NT * P                 # padded shard rows
        self.QR = N // NQ                      # table rows per slice-buffer (int16 range)
        self.SL = self.SH // NQ                # send-slice rows per core
        assert N % NC == 0 and self.SH % NQ == 0 and self.QR <= 32767
        self.PAD_IDX = pad_idx                 # -1: hw skips trailing pads; 0 for sim
        self.TB = TB                           # tiles per batch
        self.NB = (self.NT + TB - 1) // TB     # batches
        self.CT = 2                            # tiles per gather call
        self.SCRATCH = 49152                   # swdge ring bytes (ring = /16 descs)
        self.NSQ = 4                           # swdge queues (round-robin gathers)
        self.KI = IN // P                      # k chunks of layer 1
        self.KH = HID // P                     # k chunks of layer 2
        self.RB = 512                          # mlp row batch
        assert IN % P == 0 and HID % P == 0 and OUT <= P


MAXG = 1024  # hw swdge ring: max descriptors per dma_gather call


def gather_pieces(cfg, cell, off):
    """Per-batch list of gather calls [(q, slot_off, n_slots)], each <= MAXG."""
    NT, NQ = cfg.NT, cfg.NQ
    out = []
    for b in range(cfg.NB):
        bts = list(range(b * cfg.TB, min((b + 1) * cfg.TB, NT)))
        pieces = []
        for g0 in range(0, len(bts), cfg.CT):
            gts = bts[g0:g0 + cfg.CT]
            for q in range(NQ):
                for t in gts:
                    o = int(off[t, q])
                    n = int(cell[t, q])
                    p = o
                    while p < o + n:
                        m = min(MAXG, o + n - p)
                        pieces.append((q, p, m))
                        p += m
        out.append(pieces)
    return out


def preprocess(cfg, x, edge_index, W1, b1, W2, b2):
    """Host-side graph preprocessing. Returns (static schedule, per-core in_maps)."""
    N, NC, SH, NT, NQ, QR = cfg.N, cfg.NC, cfg.SH, cfg.NT, cfg.NQ, cfg.QR
    src = np.concatenate([edge_index[0], np.arange(N, dtype=np.int64)])
    dst = np.concatenate([edge_index[1], np.arange(N, dtype=np.int64)])
    deg = np.bincount(dst, minlength=N).astype(np.float64)
    dinv = 1.0 / np.sqrt(deg)
    wall = ((1.0 - cfg.alpha) * dinv[src] * dinv[dst]).astype(np.float32)

    cores = []
    cnts = np.zeros((NC, NT, NQ), np.int64)
    for c in range(NC):
        sel = (dst >= c * SH) & (dst < (c + 1) * SH)
        s_c = src[sel]
        d_c = (dst[sel] - c * SH).astype(np.int64)
        w_c = wall[sel]
        t = d_c // P
        sr = s_c // SH
        sj = s_c % SH
        q = sj // cfg.SL
        sidx = sr * cfg.SL + (sj - q * cfg.SL)   # row within slice buffer q
        o = np.lexsort((d_c, q, t))
        sidx, d_c, w_c, t, q = sidx[o], d_c[o], w_c[o], t[o], q[o]
        cnts[c] = np.bincount(t * NQ + q, minlength=NT * NQ).reshape(NT, NQ)
        cores.append((sidx, d_c, w_c))

    # static per-(tile, quarter) slot counts: max over cores, rounded to 128
    cell = ((cnts.max(axis=0) + P - 1) // P * P).astype(np.int64)  # [NT, NQ]
    cell = np.maximum(cell, P)  # at least one chunk so matmul groups are non-empty
    TOT = int(cell.sum())

    # slot offsets in (batch -> quarter -> tile) order
    off = np.zeros((NT, NQ), np.int64)
    pos = 0
    for b in range(cfg.NB):
        bts = list(range(b * cfg.TB, min((b + 1) * cfg.TB, NT)))
        for g0 in range(0, len(bts), cfg.CT):
            gts = bts[g0:g0 + cfg.CT]
            for q in range(NQ):
                for t in gts:
                    off[t, q] = pos
                    pos += int(cell[t, q])
    assert pos == TOT

    in_maps = []
    for c in range(NC):
        s_c, d_c, w_c = cores[c]
        gidx = np.full(TOT, cfg.PAD_IDX, np.int16)
        gw = np.zeros(TOT, np.float32)
        gdl = np.zeros(TOT, np.float32)
        p0 = 0
        for t in range(NT):
            for q in range(NQ):
                n = int(cnts[c, t, q])
                o = off[t, q]
                gidx[o:o + n] = s_c[p0:p0 + n].astype(np.int16)
                gw[o:o + n] = w_c[p0:p0 + n]
                gdl[o:o + n] = (d_c[p0:p0 + n] % P).astype(np.float32)
                p0 += n
        assert p0 == len(s_c)
        # idx wrap layout [16, TOT/16] per call, replicated to 128 partitions.
        # call = (batch, quarter) -> contiguous slot range
        gidx16 = np.zeros((16, TOT // 16), np.int16)
        for pieces in gather_pieces(cfg, cell, off):
            for (q, o, n) in pieces:
                blk = gidx[o:o + n].reshape(n // 16, 16).T  # j -> [j%16, j//16]
                gidx16[:, o // 16:(o + n) // 16] = blk
        gidx128 = np.tile(gidx16, (8, 1))  # replicate for 8 q7 cores
        bf16 = mybir.dt.np(mybir.dt.bfloat16)
        to_bf16 = lambda a: np.asarray(a, np.float32).astype(bf16)
        # streamed weighted one-hot: slot j=(c*128+p) -> sp[p, c*128 + dstloc_j] = w_j
        sl = np.arange(TOT)
        sp = np.zeros((TOT // P, P, P), bf16)
        sp[sl // P, sl % P, gdl.astype(np.int64)] = to_bf16(gw)
        sp128 = np.ascontiguousarray(sp.transpose(1, 0, 2).reshape(P, TOT))
        x_c = np.zeros((cfg.PSH, cfg.IN), np.float32)
        x_c[:SH] = x[c * SH:(c + 1) * SH]
        in_maps.append({
            "xt": to_bf16(np.ascontiguousarray(x_c.T)),
            "w1": to_bf16(W1),
            "b1": np.ascontiguousarray(b1.reshape(cfg.HID // P, P).T.astype(np.float32)),
            "w2": to_bf16(W2),
            "b2": to_bf16(b2.reshape(1, cfg.OUT)),
            "gidx": gidx128,
            "sp": sp128,
        })
    sched = {"cell": cell, "off": off, "TOT": TOT}
    return sched, in_maps


def build(cfg, sched):
    cell, off, TOT = sched["cell"], sched["off"], sched["TOT"]
    N, NT, NQ, QR, OUT, SH = cfg.N, cfg.NT, cfg.NQ, cfg.QR, cfg.OUT, cfg.SH
    NB, TB, K = cfg.NB, cfg.TB, cfg.K

    nc = bacc.Bacc("TRN2", target_bir_lowering=False, debug=False, num_devices=cfg.NC,
                   dynamic_dma_scratch_size=cfg.SCRATCH, num_swdge_queues=cfg.NSQ)

    xt_d = nc.declare_dram_parameter("xt", [cfg.IN, cfg.PSH], BF16, isOutput=False)
    w1_d = nc.declare_dram_parameter("w1", [cfg.IN, cfg.HID], BF16, isOutput=False)
    b1_d = nc.declare_dram_parameter("b1", [P, cfg.KH], F32, isOutput=False)
    w2_d = nc.declare_dram_parameter("w2", [cfg.HID, OUT], BF16, isOutput=False)
    b2_d = nc.declare_dram_parameter("b2", [1, OUT], BF16, isOutput=False)
    gidx_d = nc.declare_dram_parameter("gidx", [P, TOT // 16], I16, isOutput=False)
    sp_d = nc.declare_dram_parameter("sp", [P, TOT], BF16, isOutput=False)
    out_d = nc.declare_dram_parameter("out", [SH, OUT], F32, isOutput=True)

    NQ_, QR_, SL = cfg.NQ, cfg.QR, cfg.SL
    # double-buffered sliced gather tables: par in {0,1} x slice q
    table = nc.dram_tensor("table", [2 * NQ_ * QR_, OUT], F32, addr_space="Shared")
    send = nc.dram_tensor("send", [SH, OUT], F32)

    def tslice(par, q):
        o = (par * NQ_ + q) * QR_
        return table[o:o + QR_, :]

    # per-batch call sizes (slots) and chunk schedule
    allpieces = gather_pieces(cfg, cell, off)
    batches = []
    for b in range(NB):
        ts = list(range(b * TB, min((b + 1) * TB, NT)))
        regions = []   # (q, slot_off, n): contiguous (group, quarter) spans for cast/S
        chunks = []
        col = 0
        for g0 in range(0, len(ts), cfg.CT):
            gts = ts[g0:g0 + cfg.CT]
            for q in range(NQ):
                n = int(cell[gts[0]:gts[-1] + 1, q].sum())
                regions.append((q, int(off[gts[0], q]), n))
                for t in gts:
                    for j in range(int(cell[t, q]) // P):
                        chunks.append((col, t - b * TB))
                        col += 1
        last_of_tile = {}
        for k_, (col_, i) in enumerate(chunks):
            last_of_tile[i] = k_
        batches.append((ts, allpieces[b], regions, chunks, last_of_tile))
    maxreg = max(n for (_, _, regions, _, _) in batches for (_, _, n) in regions)
    maxbatch = max(sum(n for (_, _, n) in regions) for (_, _, regions, _, _) in batches)

    with tile.TileContext(nc) as tc:
        with tc.tile_pool(name="const", bufs=1) as cp:
            iota = cp.tile([P, P], BF16, tag="iota")
            iota_i = cp.tile([P, P], I32, tag="iota_i")
            nc.gpsimd.iota(iota_i[:], pattern=[[1, P]], base=0, channel_multiplier=0)
            nc.vector.tensor_copy(iota[:], iota_i[:])
            iop = cp.tile([P, 1], I32, tag="iop")
            nc.gpsimd.iota(iop[:], pattern=[[0, 1]], base=0, channel_multiplier=1)
            iopb = cp.tile([P, 1], BF16, tag="iopb")
            nc.vector.tensor_copy(iopb[:], iop[:])
            ident = cp.tile([P, P], BF16, tag="ident")
            nc.vector.tensor_tensor(
                out=ident[:], in0=iopb[:].to_broadcast([P, P]), in1=iota[:],
                op=ALU.is_equal)
            ones = cp.tile([1, P], BF16, tag="ones")
            nc.gpsimd.memset(ones[:], 1.0)

            w1s = cp.tile([P, cfg.KI, cfg.HID], BF16, tag="w1s")
            nc.sync.dma_start(w1s[:], w1_d.ap().rearrange("(k p) h -> p k h", p=P))
            w2s = cp.tile([P, cfg.KH, OUT], BF16, tag="w2s")
            nc.sync.dma_start(w2s[:], w2_d.ap().rearrange("(k p) o -> p k o", p=P))
            b1s = cp.tile([P, cfg.KH], F32, tag="b1s")
            nc.sync.dma_start(b1s[:], b1_d[:, :])
            b2s = cp.tile([1, OUT], BF16, tag="b2s")
            nc.sync.dma_start(b2s[:], b2_d[:, :])


            h0t = cp.tile([P, NT * OUT], BF16, tag="h0t")    # 0.1 * h0, bf16
            hnew = cp.tile([P, NT * OUT], F32, tag="hnew")   # current h, f32

            # ---------------- MLP head ----------------
            with (
                tc.tile_pool(name="mxt", bufs=2 * cfg.KI) as mxt,
                tc.tile_pool(name="mp1", bufs=2, space="PSUM") as mp1,
                tc.tile_pool(name="mh1", bufs=2) as mh1,
                tc.tile_pool(name="mp2", bufs=2, space="PSUM") as mp2,
            ):
                RB = cfg.RB
                nrb = (cfg.PSH + RB - 1) // RB
                for b in range(nrb):
                    r0 = b * RB
                    rows = min(RB, cfg.PSH - r0)
                    xts = []
                    for k_ in range(cfg.KI):
                        xk = mxt.tile([P, RB], BF16, tag="xt")
                        nc.sync.dma_start(xk[:, :rows], xt_d[k_ * P:(k_ + 1) * P, r0:r0 + rows])
                        xts.append(xk)
                    h1t = mh1.tile([P, cfg.KH, RB], BF16, tag="h1t")
                    for fc in range(cfg.KH):
                        ps = mp1.tile([P, RB], F32, tag=f"mp1_{fc}")
                        for k_ in range(cfg.KI):
                            nc.tensor.matmul(
                                ps[:, :rows],
                                lhsT=w1s[:, k_, fc * P:(fc + 1) * P],
                                rhs=xts[k_][:, :rows],
                                start=(k_ == 0), stop=(k_ == cfg.KI - 1))
                        nc.scalar.activation(
                            h1t[:, fc, :rows], ps[:, :rows], AF.Relu,
                            bias=b1s[:, fc:fc + 1], scale=1.0)
                    for tl in range(rows // P):
                        gt = r0 // P + tl
                        p2 = mp2.tile([P, OUT], F32, tag="mp2")
                        for fc in range(cfg.KH):
                            nc.tensor.matmul(
                                p2[:], lhsT=h1t[:, fc, tl * P:(tl + 1) * P],
                                rhs=w2s[:, fc, :], start=(fc == 0), stop=False)
                        nc.tensor.matmul(p2[:], lhsT=ones[:1, :], rhs=b2s[:1, :],
                                         start=False, stop=True)
                        nc.scalar.activation(h0t[:, gt * OUT:(gt + 1) * OUT], p2[:],
                                             AF.Copy, bias=0.0, scale=cfg.alpha)
                        nc.vector.tensor_copy(hnew[:, gt * OUT:(gt + 1) * OUT], p2[:])

            def send_rows(r0, r1):
                # DMA hnew rows [r0, r1) (layout [128, t, 64]) -> send[r0:r1, :]
                t0 = (r0 + P - 1) // P
                t1 = r1 // P
                if r0 < t0 * P:
                    ta = r0 // P
                    nc.sync.dma_start(
                        send[r0:min(r1, (ta + 1) * P), :],
                        hnew[r0 - ta * P:min(r1 - ta * P, P),
                             ta * OUT:(ta + 1) * OUT])
                if t1 > t0:
                    nc.sync.dma_start(
                        send.ap()[t0 * P:t1 * P, :].rearrange(
                            "(t p) d -> p t d", p=P),
                        hnew[:, t0 * OUT:t1 * OUT].rearrange(
                            "p (t d) -> p t d", d=OUT))
                if r1 > t1 * P and t1 * P >= r0:
                    nc.sync.dma_start(
                        send[t1 * P:r1, :],
                        hnew[0:r1 - t1 * P, t1 * OUT:(t1 + 1) * OUT])

            def slice_collective(g, par):
                send_rows(g * SL, (g + 1) * SL)
                nc.gpsimd.collective_compute(
                    "AllGather", ALU.bypass,
                    ins=[send.ap()[g * SL:(g + 1) * SL, :].opt()],
                    outs=[tslice(par, g).opt()],
                    replica_groups=[list(range(cfg.NC))])

            for g in range(NQ_):
                slice_collective(g, 0)

            # batch index -> slices whose send rows are complete after it
            slice_ready_batch = {}
            for g in range(NQ_):
                b = ((g + 1) * SL - 1) // P // TB
                slice_ready_batch.setdefault(b, []).append(g)

            # ---------------- propagation ----------------
            with (
                tc.tile_pool(name="ixp", bufs=3) as ixp,
                tc.tile_pool(name="gbp", bufs=8) as gbp,
                tc.tile_pool(name="msgp", bufs=3) as msgp,
                tc.tile_pool(name="spp", bufs=4) as spp,
                tc.tile_pool(name="pp", bufs=2, space="PSUM") as pp,
            ):
                for i_ in range(8):
                    gbz = gbp.tile([P, maxreg // P, OUT], F32, tag="gb")
                    nc.vector.memset(gbz[:], 0.0)
                qrr = [0]
                for s in range(K):
                    par = s % 2
                    for bi, (ts, pieces, regions, chunks, last_of_tile) in enumerate(batches):
                        b0 = int(off[ts[0], 0])           # first slot of batch
                        cols_b = sum(n for (_, _, n) in regions)
                        ix = ixp.tile([P, maxbatch // 16], I16, tag="ix")
                        nc.sync.dma_start(ix[:, :cols_b // 16],
                                          gidx_d[:, b0 // 16:(b0 + cols_b) // 16])
                        pss = []
                        for i, t in enumerate(ts):
                            ps = pp.tile([P, OUT], F32, tag=f"ps{i}")
                            nc.tensor.matmul(ps[:], lhsT=ident[:],
                                             rhs=h0t[:, t * OUT:(t + 1) * OUT],
                                             start=True, stop=False)
                            pss.append(ps)
                        msg = msgp.tile([P, maxbatch // P * OUT], BF16, tag="msg")
                        pi = 0
                        mcol = 0
                        for (q, o, n) in regions:
                            C = n // P
                            gb = gbp.tile([P, maxreg // P, OUT], F32, tag="gb")
                            while pi < len(pieces) and pieces[pi][1] < o + n:
                                (pq, po, pn) = pieces[pi]
                                assert pq == q and po >= o and po + pn <= o + n
                                c0 = (po - o) // P
                                nc.gpsimd.dma_gather(
                                    out_ap=gb[:, c0:c0 + pn // P, :],
                                    in_ap=tslice(par, q),
                                    idxs_ap=ix[:, (po - b0) // 16:(po - b0 + pn) // 16],
                                    num_idxs=pn, num_idxs_reg=pn, elem_size=OUT,
                                    queue_num=qrr[0] % cfg.NSQ)
                                qrr[0] += 1
                                pi += 1
                            nc.scalar.activation(
                                msg[:, mcol * OUT:(mcol + C) * OUT],
                                gb[:, :C, :].rearrange("p c d -> p (c d)"),
                                AF.Copy, bias=0.0, scale=1.0)
                            S = spp.tile([P, maxreg // P, P], BF16, tag="S")
                            nc.sync.dma_start(
                                S[:, :C, :].rearrange("p c d -> p (c d)"),
                                sp_d[:, o:o + n])
                            # chunks of this region, in order
                            for j in range(C):
                                k_ = mcol + j
                                col, i = chunks[k_]
                                assert col == k_
                                nc.tensor.matmul(
                                    pss[i][:], lhsT=S[:, j, :],
                                    rhs=msg[:, k_ * OUT:(k_ + 1) * OUT],
                                    start=False, stop=(last_of_tile[i] == k_))
                            mcol += C
                        for i, t in enumerate(ts):
                            nc.vector.tensor_copy(hnew[:, t * OUT:(t + 1) * OUT],
                                                  pss[i][:])
                        if s < K - 1:
                            for g in slice_ready_batch.get(bi, []):
                                slice_collective(g, 1 - par)

            full = (NT - 1) * P
            nc.sync.dma_start(
                out_d.ap()[0:full, :].rearrange("(t p) d -> p t d", p=P),
                hnew[:, 0:(NT - 1) * OUT].rearrange("p (t d) -> p t d", d=OUT))
            nc.sync.dma_start(out_d[full:SH, :],
                              hnew[0:SH - full, (NT - 1) * OUT:NT * OUT])

    nc.compile()
    return nc


_CACHE = {}


def kernel(x, edge_index, W1, b1, W2, b2):
    x = np.asarray(x, np.float32)
    edge_index = np.asarray(edge_index)
    W1 = np.asarray(W1, np.float32)
    b1 = np.asarray(b1, np.float32)
    W2 = np.asarray(W2, np.float32)
    b2 = np.asarray(b2, np.float32)
    cfg = Cfg(N=x.shape[0], E=edge_index.shape[1], IN=x.shape[1],
              HID=W1.shape[1], OUT=W2.shape[1])
    sched, in_maps = preprocess(cfg, x, edge_index, W1, b1, W2, b2)
    key = ("k", cfg.N, cfg.E, sched["TOT"])
    if key not in _CACHE:
        _CACHE[key] = build(cfg, sched)
    nc = _CACHE[key]
    res = bass_utils.run_bass_kernel_spmd(nc, in_maps, core_ids=list(range(cfg.NC)))
    return np.concatenate([res.results[c]["out"] for c in range(cfg.NC)], axis=0)


if __name__ == "__main__":
    pass


# revision 23
# speedup vs baseline: 3.6430x; 1.1845x over previous
"""APPNP (GNN message passing) distributed Bass kernel for 8 TRN2 NeuronCores.

Strategy (graph/data parallel, "pull" form):
  - Nodes sharded by contiguous id range across 8 cores (12500 each).
  - MLP head computed on-device per shard (xT passed pre-transposed bf16).
  - Each propagation step: AllGather the full f32 h table (100000 x 64) into
    per-core DRAM, then each core gathers h[src] rows for its dst-range edges
    via dma_gather (int16 indices -> table split into 4 row-quarters), scales
    by edge norm via a weighted one-hot (DVE is_equal + mul), and scatter-adds
    into per-tile PSUM accumulators with TensorEngine matmuls
    (agg_tile += S'^T @ msg).  Teleport term enters PSUM as an extra
    identity-matmul with 0.1*h0 (bf16).
  - Edge structure (sort, cells, padding, norms) is preprocessed on host;
    slot layout is identical on all 8 cores (SPMD single graph).
"""

import sys

sys.path.insert(0, "/opt/trn_rl_repo")

import numpy as np
import concourse.bass as bass
import concourse.bacc as bacc
import concourse.tile as tile
import concourse.mybir as mybir
from concourse import bass_utils

# SWDGE queue <-> DMASW sem-lane consistency: Tile assigns DMASW lanes by a
# global round-robin that ignores queue_num; with multi-queue dma_gather the
# (lane, queue) pairing then depends on schedule order and the sim/HW rejects
# cross-queue updates. Pin gather lanes to 2*queue_num + per-queue toggle.
import concourse.tile_sem_assignment as _tsa


def _patch_tick():
    if getattr(_tsa.TileClockTick, "_gnn_patched", False):
        return
    orig = _tsa.TileClockTick._assign_tick

    def _assign_tick(self, inst):
        if isinstance(inst, mybir.InstDMAGatherAnt):
            qn = int(getattr(inst, "queue_num", 0) or 0)
            tog = getattr(self, "_gnn_qtoggle", None)
            if tog is None:
                tog = self._gnn_qtoggle = {}
            lane = 2 * qn + tog.get(qn, 0)
            tog[qn] = 1 - tog.get(qn, 0)
            save = self.next_sw_dma_idx
            self.next_sw_dma_idx = lane
            try:
                return orig(self, inst)
            finally:
                self.next_sw_dma_idx = save
        return orig(self, inst)

    _tsa.TileClockTick._assign_tick = _assign_tick
    _tsa.TileClockTick._gnn_patched = True


_patch_tick()

F32 = mybir.dt.float32
BF16 = mybir.dt.bfloat16
I16 = mybir.dt.int16
I32 = mybir.dt.int32
AF = mybir.ActivationFunctionType
ALU = mybir.AluOpType

P = 128


class Cfg:
    def __init__(self, N=100000, E=3200000, IN=512, HID=256, OUT=64, K=10,
                 alpha=0.1, NC=8, NQ=5, TB=4, pad_idx=-1):
        self.N, self.E, self.IN, self.HID, self.OUT, self.K = N, E, IN, HID, OUT, K
        self.alpha, self.NC, self.NQ = alpha, NC, NQ
        self.SH = N // NC                      # nodes per core
        self.NT = (self.SH + P - 1) // P       # dst tiles per core
        self.PSH = self.NT * P                 # padded shard rows
        self.QR = N // NQ                      # table rows per slice-buffer (int16 range)
        self.SL = self.SH // NQ                # send-slice rows per core
        assert N % NC == 0 and self.SH % NQ == 0 and self.QR <= 32767
        self.PAD_IDX = pad_idx                 # -1: hw skips trailing pads; 0 for sim
        self.TB = TB                           # tiles per batch
        self.NB = (self.NT + TB - 1) // TB     # batches
        self.CT = 2                            # tiles per gather call
        self.SCRATCH = 49152                   # swdge ring bytes (ring = /16 descs)
        self.NSQ = 4                           # swdge queues (round-robin gathers)
        self.KI = IN // P                      # k chunks of layer 1
        self.KH = HID // P                     # k chunks of layer 2
        self.RB = 512                          # mlp row batch
        assert IN % P == 0 and HID % P == 0 and OUT <= P


MAXG = 1024  # hw swdge ring: max descriptors per dma_gather call


def gather_pieces(cfg, cell, off):
    """Per-batch list of gather calls [(q, slot_off, n_slots)], each <= MAXG."""
    NT, NQ = cfg.NT, cfg.NQ
    out = []
    for b in range(cfg.NB):
        bts = list(range(b * cfg.TB, min((b + 1) * cfg.TB, NT)))
        pieces = []
        for g0 in range(0, len(bts), cfg.CT):
            gts = bts[g0:g0 + cfg.CT]
            for q in range(NQ):
                for t in gts:
                    o = int(off[t, q])
                    n = int(cell[t, q])
                    p = o
                    while p < o + n:
                        m = min(MAXG, o + n - p)
                        pieces.append((q, p, m))
                        p += m
        out.append(pieces)
    return out


def preprocess(cfg, x, edge_index, W1, b1, W2, b2):
    """Host-side graph preprocessing. Returns (static schedule, per-core in_maps)."""
    N, NC, SH, NT, NQ, QR = cfg.N, cfg.NC, cfg.SH, cfg.NT, cfg.NQ, cfg.QR
    src = np.concatenate([edge_index[0], np.arange(N, dtype=np.int64)])
    dst = np.concatenate([edge_index[1], np.arange(N, dtype=np.int64)])
    deg = np.bincount(dst, minlength=N).astype(np.float64)
    dinv = 1.0 / np.sqrt(deg)
    wall = ((1.0 - cfg.alpha) * dinv[src] * dinv[dst]).astype(np.float32)

    cores = []
    cnts = np.zeros((NC, NT, NQ), np.int64)
    for c in range(NC):
        sel = (dst >= c * SH) & (dst < (c + 1) * SH)
        s_c = src[sel]
        d_c = (dst[sel] - c * SH).astype(np.int64)
        w_c = wall[sel]
        t = d_c // P
        sr = s_c // SH
        sj = s_c % SH
        q = sj // cfg.SL
        sidx = sr * cfg.SL + (sj - q * cfg.SL)   # row within slice buffer q
        o = np.lexsort((d_c, q, t))
        sidx, d_c, w_c, t, q = sidx[o], d_c[o], w_c[o], t[o], q[o]
        cnts[c] = np.bincount(t * NQ + q, minlength=NT * NQ).reshape(NT, NQ)
        cores.append((sidx, d_c, w_c))

    # static per-(tile, quarter) slot counts: max over cores, rounded to 128
    cell = ((cnts.max(axis=0) + P - 1) // P * P).astype(np.int64)  # [NT, NQ]
    cell = np.maximum(cell, P)  # at least one chunk so matmul groups are non-empty
    TOT = int(cell.sum())

    # slot offsets in (batch -> quarter -> tile) order
    off = np.zeros((NT, NQ), np.int64)
    pos = 0
    for b in range(cfg.NB):
        bts = list(range(b * cfg.TB, min((b + 1) * cfg.TB, NT)))
        for g0 in range(0, len(bts), cfg.CT):
            gts = bts[g0:g0 + cfg.CT]
            for q in range(NQ):
                for t in gts:
                    off[t, q] = pos
                    pos += int(cell[t, q])
    assert pos == TOT

    in_maps = []
    for c in range(NC):
        s_c, d_c, w_c = cores[c]
        gidx = np.full(TOT, cfg.PAD_IDX, np.int16)
        gw = np.zeros(TOT, np.float32)
        gdl = np.zeros(TOT, np.float32)
        p0 = 0
        for t in range(NT):
            for q in range(NQ):
                n = int(cnts[c, t, q])
                o = off[t, q]
                gidx[o:o + n] = s_c[p0:p0 + n].astype(np.int16)
                gw[o:o + n] = w_c[p0:p0 + n]
                gdl[o:o + n] = (d_c[p0:p0 + n] % P).astype(np.float32)
                p0 += n
        assert p0 == len(s_c)
        # idx wrap layout [16, n/16] per gather call, replicated to 128 partitions.
        gidx16 = np.zeros((16, TOT // 16), np.int16)
        gcnt = []
        for pieces in gather_pieces(cfg, cell, off):
            for (q, o, n) in pieces:
                if gidx[o] < 0:
                    gidx[o] = 0  # keep >=1 valid idx per call (w=0 slot)
                blk = gidx[o:o + n].reshape(n // 16, 16).T  # j -> [j%16, j//16]
                gidx16[:, o // 16:(o + n) // 16] = blk
                gcnt.append(int((gidx[o:o + n] >= 0).sum()))
        gidx128 = np.tile(gidx16, (8, 1))  # replicate for 8 q7 cores
        bf16 = mybir.dt.np(mybir.dt.bfloat16)
        to_bf16 = lambda a: np.asarray(a, np.float32).astype(bf16)
        # streamed weighted one-hot: slot j=(c*128+p) -> sp[p, c*128 + dstloc_j] = w_j
        sl = np.arange(TOT)
        sp = np.zeros((TOT // P, P, P), bf16)
        sp[sl // P, sl % P, gdl.astype(np.int64)] = to_bf16(gw)
        sp128 = np.ascontiguousarray(sp.transpose(1, 0, 2).reshape(P, TOT))
        x_c = np.zeros((cfg.PSH, cfg.IN), np.float32)
        x_c[:SH] = x[c * SH:(c + 1) * SH]
        in_maps.append({
            "xt": to_bf16(np.ascontiguousarray(x_c.T)),
            "w1": to_bf16(W1),
            "b1": np.ascontiguousarray(b1.reshape(cfg.HID // P, P).T.astype(np.float32)),
            "w2": to_bf16(W2),
            "b2": to_bf16(b2.reshape(1, cfg.OUT)),
            "gidx": gidx128,
            "gcnt": np.asarray(gcnt, np.int32).reshape(1, -1),
            "sp": sp128,
        })
    sched = {"cell": cell, "off": off, "TOT": TOT}
    return sched, in_maps


def build(cfg, sched):
    cell, off, TOT = sched["cell"], sched["off"], sched["TOT"]
    N, NT, NQ, QR, OUT, SH = cfg.N, cfg.NT, cfg.NQ, cfg.QR, cfg.OUT, cfg.SH
    NB, TB, K = cfg.NB, cfg.TB, cfg.K

    nc = bacc.Bacc("TRN2", target_bir_lowering=False, debug=False, num_devices=cfg.NC,
                   dynamic_dma_scratch_size=cfg.SCRATCH, num_swdge_queues=cfg.NSQ)

    xt_d = nc.declare_dram_parameter("xt", [cfg.IN, cfg.PSH], BF16, isOutput=False)
    w1_d = nc.declare_dram_parameter("w1", [cfg.IN, cfg.HID], BF16, isOutput=False)
    b1_d = nc.declare_dram_parameter("b1", [P, cfg.KH], F32, isOutput=False)
    w2_d = nc.declare_dram_parameter("w2", [cfg.HID, OUT], BF16, isOutput=False)
    b2_d = nc.declare_dram_parameter("b2", [1, OUT], BF16, isOutput=False)
    gidx_d = nc.declare_dram_parameter("gidx", [P, TOT // 16], I16, isOutput=False)
    ncalls = sum(len(p) for p in gather_pieces(cfg, cell, off))
    gcnt_d = nc.declare_dram_parameter("gcnt", [1, ncalls], I32, isOutput=False)
    sp_d = nc.declare_dram_parameter("sp", [P, TOT], BF16, isOutput=False)
    out_d = nc.declare_dram_parameter("out", [SH, OUT], F32, isOutput=True)

    NQ_, QR_, SL = cfg.NQ, cfg.QR, cfg.SL
    # double-buffered sliced gather tables: par in {0,1} x slice q
    table = nc.dram_tensor("table", [2 * NQ_ * QR_, OUT], F32, addr_space="Shared")
    send = nc.dram_tensor("send", [SH, OUT], F32)

    def tslice(par, q):
        o = (par * NQ_ + q) * QR_
        return table[o:o + QR_, :]

    # per-batch call sizes (slots) and chunk schedule
    allpieces = gather_pieces(cfg, cell, off)
    batches = []
    for b in range(NB):
        ts = list(range(b * TB, min((b + 1) * TB, NT)))
        regions = []   # (q, slot_off, n): contiguous (group, quarter) spans for cast/S
        chunks = []
        col = 0
        for g0 in range(0, len(ts), cfg.CT):
            gts = ts[g0:g0 + cfg.CT]
            for q in range(NQ):
                n = int(cell[gts[0]:gts[-1] + 1, q].sum())
                regions.append((q, int(off[gts[0], q]), n))
                for t in gts:
                    for j in range(int(cell[t, q]) // P):
                        chunks.append((col, t - b * TB))
                        col += 1
        last_of_tile = {}
        for k_, (col_, i) in enumerate(chunks):
            last_of_tile[i] = k_
        batches.append((ts, allpieces[b], regions, chunks, last_of_tile))
    maxreg = max(n for (_, _, regions, _, _) in batches for (_, _, n) in regions)
    maxbatch = max(sum(n for (_, _, n) in regions) for (_, _, regions, _, _) in batches)

    with tile.TileContext(nc) as tc:
        with tc.tile_pool(name="const", bufs=1) as cp:
            iota = cp.tile([P, P], BF16, tag="iota")
            iota_i = cp.tile([P, P], I32, tag="iota_i")
            nc.gpsimd.iota(iota_i[:], pattern=[[1, P]], base=0, channel_multiplier=0)
            nc.vector.tensor_copy(iota[:], iota_i[:])
            iop = cp.tile([P, 1], I32, tag="iop")
            nc.gpsimd.iota(iop[:], pattern=[[0, 1]], base=0, channel_multiplier=1)
            iopb = cp.tile([P, 1], BF16, tag="iopb")
            nc.vector.tensor_copy(iopb[:], iop[:])
            ident = cp.tile([P, P], BF16, tag="ident")
            nc.vector.tensor_tensor(
                out=ident[:], in0=iopb[:].to_broadcast([P, P]), in1=iota[:],
                op=ALU.is_equal)
            ones = cp.tile([1, P], BF16, tag="ones")
            nc.gpsimd.memset(ones[:], 1.0)

            w1s = cp.tile([P, cfg.KI, cfg.HID], BF16, tag="w1s")
            nc.sync.dma_start(w1s[:], w1_d.ap().rearrange("(k p) h -> p k h", p=P))
            w2s = cp.tile([P, cfg.KH, OUT], BF16, tag="w2s")
            nc.sync.dma_start(w2s[:], w2_d.ap().rearrange("(k p) o -> p k o", p=P))
            b1s = cp.tile([P, cfg.KH], F32, tag="b1s")
            nc.sync.dma_start(b1s[:], b1_d[:, :])
            b2s = cp.tile([1, OUT], BF16, tag="b2s")
            nc.sync.dma_start(b2s[:], b2_d[:, :])


            gcnt_t = cp.tile([1, ncalls], I32, tag="gcnt")
            nc.sync.dma_start(gcnt_t[:], gcnt_d[:, :])
            from concourse import mybir as _mb
            cntregs = [nc.alloc_register(_mb.EngineType.Pool, f"gcntr{i}")
                       for i in range(8)]
            h0t = cp.tile([P, NT * OUT], BF16, tag="h0t")    # 0.1 * h0, bf16
            hnew = cp.tile([P, NT * OUT], F32, tag="hnew")   # current h, f32

            # ---------------- MLP head ----------------
            with (
                tc.tile_pool(name="mxt", bufs=2 * cfg.KI) as mxt,
                tc.tile_pool(name="mp1", bufs=2, space="PSUM") as mp1,
                tc.tile_pool(name="mh1", bufs=2) as mh1,
                tc.tile_pool(name="mp2", bufs=2, space="PSUM") as mp2,
            ):
                RB = cfg.RB
                nrb = (cfg.PSH + RB - 1) // RB
                for b in range(nrb):
                    r0 = b * RB
                    rows = min(RB, cfg.PSH - r0)
                    xts = []
                    for k_ in range(cfg.KI):
                        xk = mxt.tile([P, RB], BF16, tag="xt")
                        nc.sync.dma_start(xk[:, :rows], xt_d[k_ * P:(k_ + 1) * P, r0:r0 + rows])
                        xts.append(xk)
                    h1t = mh1.tile([P, cfg.KH, RB], BF16, tag="h1t")
                    for fc in range(cfg.KH):
                        ps = mp1.tile([P, RB], F32, tag=f"mp1_{fc}")
                        for k_ in range(cfg.KI):
                            nc.tensor.matmul(
                                ps[:, :rows],
                                lhsT=w1s[:, k_, fc * P:(fc + 1) * P],
                                rhs=xts[k_][:, :rows],
                                start=(k_ == 0), stop=(k_ == cfg.KI - 1))
                        nc.scalar.activation(
                            h1t[:, fc, :rows], ps[:, :rows], AF.Relu,
                            bias=b1s[:, fc:fc + 1], scale=1.0)
                    for tl in range(rows // P):
                        gt = r0 // P + tl
                        p2 = mp2.tile([P, OUT], F32, tag="mp2")
                        for fc in range(cfg.KH):
                            nc.tensor.matmul(
                                p2[:], lhsT=h1t[:, fc, tl * P:(tl + 1) * P],
                                rhs=w2s[:, fc, :], start=(fc == 0), stop=False)
                        nc.tensor.matmul(p2[:], lhsT=ones[:1, :], rhs=b2s[:1, :],
                                         start=False, stop=True)
                        nc.scalar.activation(h0t[:, gt * OUT:(gt + 1) * OUT], p2[:],
                                             AF.Copy, bias=0.0, scale=cfg.alpha)
                        nc.vector.tensor_copy(hnew[:, gt * OUT:(gt + 1) * OUT], p2[:])

            def send_rows(r0, r1):
                # DMA hnew rows [r0, r1) (layout [128, t, 64]) -> send[r0:r1, :]
                t0 = (r0 + P - 1) // P
                t1 = r1 // P
                if r0 < t0 * P:
                    ta = r0 // P
                    nc.sync.dma_start(
                        send[r0:min(r1, (ta + 1) * P), :],
                        hnew[r0 - ta * P:min(r1 - ta * P, P),
                             ta * OUT:(ta + 1) * OUT])
                if t1 > t0:
                    nc.sync.dma_start(
                        send.ap()[t0 * P:t1 * P, :].rearrange(
                            "(t p) d -> p t d", p=P),
                        hnew[:, t0 * OUT:t1 * OUT].rearrange(
                            "p (t d) -> p t d", d=OUT))
                if r1 > t1 * P and t1 * P >= r0:
                    nc.sync.dma_start(
                        send[t1 * P:r1, :],
                        hnew[0:r1 - t1 * P, t1 * OUT:(t1 + 1) * OUT])

            def slice_collective(g, par):
                send_rows(g * SL, (g + 1) * SL)
                nc.gpsimd.collective_compute(
                    "AllGather", ALU.bypass,
                    ins=[send.ap()[g * SL:(g + 1) * SL, :].opt()],
                    outs=[tslice(par, g).opt()],
                    replica_groups=[list(range(cfg.NC))])

            for g in range(NQ_):
                slice_collective(g, 0)

            # batch index -> slices whose send rows are complete after it
            slice_ready_batch = {}
            for g in range(NQ_):
                b = ((g + 1) * SL - 1) // P // TB
                slice_ready_batch.setdefault(b, []).append(g)

            # ---------------- propagation ----------------
            with (
                tc.tile_pool(name="ixp", bufs=3) as ixp,
                tc.tile_pool(name="gbp", bufs=1) as gbp,
                tc.tile_pool(name="msgp", bufs=3) as msgp,
                tc.tile_pool(name="spp", bufs=4) as spp,
                tc.tile_pool(name="pp", bufs=2, space="PSUM") as pp,
            ):
                gbs = []
                for i_ in range(8):
                    gbt = gbp.tile([P, maxreg // P, OUT], F32, tag=f"gb{i_}",
                                   name=f"gbt{i_}")
                    gbs.append(gbt)
                    nc.vector.memset(gbt[:], 0.0)
                qrr = [0]
                grr = [0]
                ci = [0]
                for s in range(K):
                    par = s % 2
                    for bi, (ts, pieces, regions, chunks, last_of_tile) in enumerate(batches):
                        b0 = int(off[ts[0], 0])           # first slot of batch
                        cols_b = sum(n for (_, _, n) in regions)
                        ix = ixp.tile([P, maxbatch // 16], I16, tag="ix")
                        nc.sync.dma_start(ix[:, :cols_b // 16],
                                          gidx_d[:, b0 // 16:(b0 + cols_b) // 16])
                        pss = []
                        for i, t in enumerate(ts):
                            ps = pp.tile([P, OUT], F32, tag=f"ps{i}")
                            nc.tensor.matmul(ps[:], lhsT=ident[:],
                                             rhs=h0t[:, t * OUT:(t + 1) * OUT],
                                             start=True, stop=False)
                            pss.append(ps)
                        msg = msgp.tile([P, maxbatch // P * OUT], BF16, tag="msg")
                        pi = 0
                        mcol = 0
                        for (q, o, n) in regions:
                            C = n // P
                            gb = gbs[grr[0] % 8]
                            grr[0] += 1
                            while pi < len(pieces) and pieces[pi][1] < o + n:
                                (pq, po, pn) = pieces[pi]
                                assert pq == q and po >= o and po + pn <= o + n
                                c0 = (po - o) // P
                                creg = cntregs[qrr[0] % 8]
                                nc.gpsimd.reg_load(
                                    creg, gcnt_t[0:1, ci[0] % ncalls:ci[0] % ncalls + 1])
                                nc.gpsimd.dma_gather(
                                    out_ap=gb[:, c0:c0 + pn // P, :],
                                    in_ap=tslice(par, q),
                                    idxs_ap=ix[:, (po - b0) // 16:(po - b0 + pn) // 16],
                                    num_idxs=pn, num_idxs_reg=creg, elem_size=OUT,
                                    queue_num=qrr[0] % cfg.NSQ)
                                qrr[0] += 1
                                ci[0] += 1
                                pi += 1
                            nc.scalar.activation(
                                msg[:, mcol * OUT:(mcol + C) * OUT],
                                gb[:, :C, :].rearrange("p c d -> p (c d)"),
                                AF.Copy, bias=0.0, scale=1.0)
                            S = spp.tile([P, maxreg // P, P], BF16, tag="S")
                            nc.sync.dma_start(
                                S[:, :C, :].rearrange("p c d -> p (c d)"),
                                sp_d[:, o:o + n])
                            # chunks of this region, in order
                            for j in range(C):
                                k_ = mcol + j
                                col, i = chunks[k_]
                                assert col == k_
                                nc.tensor.matmul(
                                    pss[i][:], lhsT=S[:, j, :],
                                    rhs=msg[:, k_ * OUT:(k_ + 1) * OUT],
                                    start=False, stop=(last_of_tile[i] == k_))
                            mcol += C
                        for i, t in enumerate(ts):
                            nc.vector.tensor_copy(hnew[:, t * OUT:(t + 1) * OUT],
                                                  pss[i][:])
                        if s < K - 1:
                            for g in slice_ready_batch.get(bi, []):
                                slice_collective(g, 1 - par)

            full = (NT - 1) * P
            nc.sync.dma_start(
                out_d.ap()[0:full, :].rearrange("(t p) d -> p t d", p=P),
                hnew[:, 0:(NT - 1) * OUT].rearrange("p (t d) -> p t d", d=OUT))
            nc.sync.dma_start(out_d[full:SH, :],
                              hnew[0:SH - full, (NT - 1) * OUT:NT * OUT])

    nc.compile()
    return nc


_CACHE = {}


def kernel(x, edge_index, W1, b1, W2, b2):
    x = np.asarray(x, np.float32)
    edge_index = np.asarray(edge_index)
    W1 = np.asarray(W1, np.float32)
    b1 = np.asarray(b1, np.float32)
    W2 = np.asarray(W2, np.float32)
    b2 = np.asarray(b2, np.float32)
    cfg = Cfg(N=x.shape[0], E=edge_index.shape[1], IN=x.shape[1],
              HID=W1.shape[1], OUT=W2.shape[1])
    sched, in_maps = preprocess(cfg, x, edge_index, W1, b1, W2, b2)
    key = ("k", cfg.N, cfg.E, sched["TOT"])
    if key not in _CACHE:
        _CACHE[key] = build(cfg, sched)
    nc = _CACHE[key]
    res = bass_utils.run_bass_kernel_spmd(nc, in_maps, core_ids=list(range(cfg.NC)))
    return np.concatenate([res.results[c]["out"] for c in range(cfg.NC)], axis=0)


if __name__ == "__main__":
    pass
